# revision 23
# baseline (speedup 1.0000x reference)
"""Distributed causal multi-head attention for Trainium2 (8 NeuronCores).

Problem (hardcoded): x[2, 2048, 1024], 16 heads, head_dim 64, causal
softmax(QK^T/8)V then out-proj with bias. f32 in/out.

Sharding: data parallel on batch (cores 0-3 -> batch 0, 4-7 -> batch 1),
tensor parallel on heads within each group of 4 (4 heads per core).
Each core:
  - computes Q^T,K^T via fp8(e4m3) DoubleRow matmuls (x pre-scaled by 8,
    Wq/Wk by 64 on the host; the 512^2 product scale is folded into the
    softmax exp scale), V in bf16
  - scores transposed S^T[k,q] = K Q^T so the softmax denominator comes out
    of the PE via an appended ones-column on V (no partition reductions)
  - exp without max-subtraction (scores are O(2), safe in fp32/bf16)
  - causal mask applied post-exp as a 0/1 bf16 multiply (DVE 2x mode)
  - ctx^T accumulated per q-chunk, normalized with 1/den partition-broadcast
  - row-parallel out-proj: each core computes the FULL-width partial
    outT_part[oc, q] = Wo[own 256 rows, oc]^T ctxT_own
No collectives: the host sums the 4 partial outputs per batch group
(standard row-parallel TP unshard), adds the bias, and transposes.

Attention/out-proj matmuls bf16 (fp32 PSUM accumulation); QK projections
fp8; partial outputs written bf16.
"""

import numpy as np
import ml_dtypes

from concourse import bass, bacc, mybir
from concourse import tile
from concourse.bass_utils import run_bass_kernel_spmd

BF16 = mybir.dt.bfloat16
F32 = mybir.dt.float32
FP8 = mybir.dt.float8e4
Act = mybir.ActivationFunctionType
DR = mybir.MatmulPerfMode.DoubleRow

B, S, D = 2, 2048, 1024
H, HD = 16, 64
NCORES = 8
GROUP = 4            # cores per batch group
HPC = H // GROUP     # 4 heads per core
CW = HPC * HD        # 256 ctx columns per core
QC = 512             # q-chunk width
KC = 128             # k-chunk width
NQ = S // QC         # 4
NKC = S // KC        # 16
KPQ = QC // KC       # 4 k-chunks per q-chunk
DCH = D // 128       # 8 contraction chunks of 128
CCH = CW // 128      # 2 own-ctx contraction chunks
OCH = D // 128       # 8 out-column chunks

X_SCALE = 8.0        # host pre-scale of x before fp8 quantization
W_SCALE = 64.0       # host pre-scale of Wq/Wk before fp8 quantization
# scores' = (512 q)·(512 k); exp(q·k/8) -> scale out the 512^2
EXP_SCALE = 0.125 / (X_SCALE * W_SCALE) ** 2

_CACHE = {}


def _build_bass(reps=1):
    nc = bacc.Bacc(
        "TRN2", target_bir_lowering=False, debug=False, num_devices=NCORES
    )

    # per-core external inputs, pre-staged by the host in SBUF layout
    # [partition, chunk, free] so each is a single large-descriptor DMA
    x8 = nc.declare_dram_parameter("x8", [128, DCH, S], FP8, isOutput=False)
    xT = nc.declare_dram_parameter("xT", [128, DCH, S], BF16, isOutput=False)
    wq = nc.declare_dram_parameter("wq", [128, DCH, CW], FP8, isOutput=False)
    wk = nc.declare_dram_parameter("wk", [128, DCH, CW], FP8, isOutput=False)
    wv = nc.declare_dram_parameter("wv", [128, DCH, CW], BF16, isOutput=False)
    wo = nc.declare_dram_parameter("wo", [128, CCH, D], BF16, isOutput=False)
    msk = nc.declare_dram_parameter("msk", [128, KPQ, QC], BF16, isOutput=False)
    vones = nc.declare_dram_parameter("vones", [128, NKC, HPC, 1], BF16, isOutput=False)
    # selector for den broadcast: bc[m,q] = sum_k sel33[k,m]*den_pair[k,q]
    sel33 = nc.declare_dram_parameter("sel33", [33, 128], BF16, isOutput=False)
    outT = nc.declare_dram_parameter("outT", [D, S], BF16, isOutput=True)

    with tile.TileContext(nc) as tc:
        with tc.tile_pool(name="persist", bufs=1) as pp:
            x8_sb = pp.tile([128, DCH, S], FP8, tag="x8_sb")
            xT_sb = pp.tile([128, DCH, S], BF16, tag="xT_sb")
            wq_sb = pp.tile([128, DCH, CW], FP8, tag="wq_sb")
            wk_sb = pp.tile([128, DCH, CW], FP8, tag="wk_sb")
            wv_sb = pp.tile([128, DCH, CW], BF16, tag="wv_sb")
            wo_sb = pp.tile([128, CCH, D], BF16, tag="wo_sb")
            msk_sb = pp.tile([128, KPQ, QC], BF16, tag="msk_sb")
            qT_sb = pp.tile([128, 2, S], BF16, tag="qT_sb")
            kT_sb = pp.tile([128, 2, S], BF16, tag="kT_sb")
            v_aug = pp.tile([128, NKC, HPC, HD + 1], BF16, tag="v_aug")
            ctxu0 = pp.tile([128, S], F32, tag="ctxu0")
            ctxu1 = pp.tile([128, S], F32, tag="ctxu1")
            ctxn0 = pp.tile([128, S], BF16, tag="ctxn0")
            ctxn1 = pp.tile([128, S], BF16, tag="ctxn1")
            # den per pair: head 2p at partition 0, head 2p+1 at partition
            # 32 (ACT writes must start at multiples of 32); rows 1-31 are
            # zeroed so the K=33 selector matmul can broadcast both heads
            # to output partitions 0-63 / 64-127 in one instruction
            den_pair = [pp.tile([33, S], BF16, tag=f"den{p}", name=f"den{p}")
                        for p in range(2)]
            sel_sb = pp.tile([33, 128], BF16, tag="sel_sb")
            ctxu_pair = [ctxu0, ctxu1]
            ctxn_pair = [ctxn0, ctxn1]
            # Pool engine (idle otherwise) zeroes the den scratch
            for p in range(2):
                nc.gpsimd.memset(den_pair[p][:], 0.0)

            # DMA order = first-use order. The first x8 window is split
            # per d-chunk so the very first projection chain unblocks
            # after wq + one small chunk; everything else streams behind.
            # All of x8 (fp8, 2MB) lands before xT (bf16, 4MB) so the QK
            # projections and all j=0 scores can run while V streams in.
            def _x8w(w):
                nc.sync.dma_start(
                    x8_sb[:, :, w * QC:(w + 1) * QC],
                    x8[:, :, w * QC:(w + 1) * QC],
                )

            def _xTw(w):
                nc.sync.dma_start(
                    xT_sb[:, :, w * QC:(w + 1) * QC],
                    xT[:, :, w * QC:(w + 1) * QC],
                )

            nc.sync.dma_start(wq_sb[:], wq[:])
            nc.sync.dma_start(wk_sb[:], wk[:])
            for c in range(DCH):
                nc.sync.dma_start(x8_sb[:, c, 0:QC], x8[:, c, 0:QC])
            nc.sync.dma_start(msk_sb[:], msk[:])
            _x8w(1)
            nc.sync.dma_start(wv_sb[:], wv[:])
            # ones column of V_aug comes from the host: keeps the V
            # PSUM->SBUF copy to a single (PE) sync wait
            nc.sync.dma_start(v_aug[:, :, :, HD:HD + 1], vones[:])
            nc.sync.dma_start(sel_sb[:], sel33[:])
            _xTw(0)
            _x8w(2)
            _xTw(1)
            _x8w(3)
            _xTw(2)
            _xTw(3)
            nc.sync.dma_start(wo_sb[:], wo[:])

            def _emit_once():
                with tc.tile_pool(name="proj_ps", bufs=2, space="PSUM") as projp, \
                     tc.tile_pool(name="sc_ps", bufs=2, space="PSUM") as scp, \
                     tc.tile_pool(name="ctbc_ps", bufs=2, space="PSUM") as ctp, \
                     tc.tile_pool(name="es_pool", bufs=22) as esp, \
                     tc.tile_pool(name="out_sb", bufs=8) as outs, \
                     tc.tile_pool(name="norm", bufs=2) as np_pool:

                    def qk_round(j):
                        # Q,K projections for q/k-token window j, both
                        # pairs; fp8 DoubleRow: chunk pairs -> K=256
                        for pair in range(2):
                            for w_sb, dst in ((wq_sb, qT_sb), (wk_sb, kT_sb)):
                                ps = projp.tile([128, QC], F32, tag="proj")
                                for c in range(0, DCH, 2):
                                    nc.tensor.matmul(
                                        ps[:],
                                        w_sb[:, c:c + 2, pair * 128:(pair + 1) * 128],
                                        x8_sb[:, c:c + 2, j * QC:(j + 1) * QC],
                                        start=(c == 0),
                                        stop=(c == DCH - 2),
                                        perf_mode=DR,
                                    )
                                nc.vector.tensor_copy(
                                    dst[:, pair, j * QC:(j + 1) * QC], ps[:]
                                )

                    def v_round(w):
                        # V for token chunks 4w..4w+3, both pairs
                        for t in range(4 * w, 4 * w + 4):
                            for pair in range(2):
                                ps = projp.tile([128, QC], F32, tag="proj")
                                for c in range(DCH):
                                    nc.tensor.matmul(
                                        ps[:, 0:128],
                                        xT_sb[:, c, t * 128:(t + 1) * 128],
                                        wv_sb[:, c, pair * 128:(pair + 1) * 128],
                                        start=(c == 0),
                                        stop=(c == DCH - 1),
                                    )
                                nc.vector.tensor_copy(
                                    v_aug[:, t, 2 * pair:2 * pair + 2, 0:HD],
                                    ps[:, 0:128].rearrange("p (h w) -> p h w", h=2),
                                )

                    def scores_exp(h, j):
                        """S^T then exp (+ causal masking) for q-chunk j of
                        head h. Off-band k-chunks (fully below the diagonal)
                        get full-width matmuls; the 4-chunk diagonal band
                        uses shrinking q-windows (exact block causality)
                        with a 128-wide triangle mask per chunk. Returns the
                        es tiles (off-band pairs + band tiles 1 and 2)."""
                        pair, hh = h // 2, h % 2
                        row = hh * 64
                        qs = slice(j * QC, (j + 1) * QC)
                        es_tiles = []
                        # off-band: k-chunks 0 .. 4j-1, two per PSUM tile
                        for c0 in range(0, 4 * j, 2):
                            st = scp.tile([128, 2, QC], F32, tag="st")
                            for i in range(2):
                                c = c0 + i
                                nc.tensor.matmul(
                                    st[:, i, :],
                                    kT_sb[row:row + 64, pair, c * KC:(c + 1) * KC],
                                    qT_sb[row:row + 64, pair, qs],
                                    start=True, stop=True,
                                )
                            es = esp.tile([128, 2, QC], BF16, tag="es")
                            nc.scalar.activation(es[:], st[:], Act.Exp,
                                                 scale=EXP_SCALE)
                            es_tiles.append(es)
                        # diagonal band: k-chunks 4j+r, q-window [128r, 512)
                        # packed as two tiles; sub-window starts snapped so
                        # each exp is one full-AP instruction (the unwritten
                        # PSUM slivers are exp'd but never read)
                        win = [0, KC, 2 * KC, 3 * KC]
                        for ti in range(2):
                            st = scp.tile([128, 2, QC], F32, tag="st")
                            for i in range(2):
                                r = 2 * ti + i
                                w0 = win[r]
                                nc.tensor.matmul(
                                    st[:, i, w0:QC],
                                    kT_sb[row:row + 64, pair,
                                          (4 * j + r) * KC:(4 * j + r + 1) * KC],
                                    qT_sb[row:row + 64, pair,
                                          j * QC + w0:(j + 1) * QC],
                                    start=True, stop=True,
                                )
                            es = esp.tile([128, 2, QC], BF16, tag="es")
                            lo = win[2 * ti]
                            nc.scalar.activation(es[:, :, lo:QC],
                                                 st[:, :, lo:QC],
                                                 Act.Exp, scale=EXP_SCALE)
                            # triangle mask on the leading 128 q of each
                            # chunk's window
                            for i in range(2):
                                r = 2 * ti + i
                                w0 = win[r]
                                nc.vector.tensor_mul(
                                    es[:, i, w0:w0 + KC], es[:, i, w0:w0 + KC],
                                    msk_sb[:, r, w0:w0 + KC],
                                )
                            es_tiles.append(es)
                        return es_tiles

                    def ctx_acc(h, j, es_tiles):
                        """attn@V accumulation + write-back for (h, j)."""
                        pair, hh = h // 2, h % 2
                        row = hh * 64
                        qs = slice(j * QC, (j + 1) * QC)
                        win = [0, KC, 2 * KC, 3 * KC]
                        ct = ctp.tile([HD + 1, QC], F32, tag="ct")
                        for c in range(4 * j):
                            nc.tensor.matmul(
                                ct[:],
                                v_aug[:, c, h, :],
                                es_tiles[c // 2][:, c % 2, :],
                                start=(c == 0),
                                stop=False,
                            )
                        for r in range(4):
                            w0 = win[r]
                            nc.tensor.matmul(
                                ct[:, w0:QC],
                                v_aug[:, 4 * j + r, h, :],
                                es_tiles[2 * j + r // 2][:, r % 2, w0:QC],
                                start=(j == 0 and r == 0),
                                stop=(r == 3),
                            )
                        nc.vector.tensor_copy(
                            ctxu_pair[pair][row:row + 64, qs], ct[0:HD, :]
                        )
                        nc.vector.tensor_copy(
                            den_pair[pair][hh * 32:hh * 32 + 1, qs],
                            ct[HD:HD + 1, :],
                        )

                    def norm(pair, j):
                        # den[pair][:, qs] complete once both heads of the
                        # pair finished ctx for q-chunk j
                        qs = slice(j * QC, (j + 1) * QC)
                        bc = ctp.tile([128, QC], F32, tag="ct")
                        nc.tensor.matmul(
                            bc[:], sel_sb[:], den_pair[pair][:, qs],
                            start=True, stop=True,
                        )
                        rb = np_pool.tile([128, QC], F32, tag="rb")
                        nc.vector.reciprocal(rb[:], bc[:])
                        nc.vector.tensor_mul(
                            ctxn_pair[pair][:, qs],
                            ctxu_pair[pair][:, qs], rb[:],
                        )

                    # Interleaved emission. QK projections and all j=0
                    # scores depend only on x8 (fp8, lands first), so they
                    # run while the larger bf16 x for V is still streaming;
                    # exp starts ~15us earlier than a phase-ordered kernel.
                    # The attention pipeline emits scores of the next chunk
                    # before ctx of the previous one so the PE has queued
                    # work while ACT runs exp; each (pair, j) normalizes as
                    # soon as its den is complete.
                    def out_round(j, last=False):
                        # row-parallel out-proj for q-chunk j:
                        # outT_part[oc, q] = Wo[own, oc]^T ctxn_own (bias on
                        # host). PSUM alternates between the projection pool
                        # (dead once the v_rounds finish) and a scores-pool
                        # slice so the matmuls can run ~2 copies ahead.
                        # While interleaved into late attention (ACT is
                        # exp-critical there) all copies go to DVE with the
                        # DMA issued from the idle Pool queue; the final
                        # round alternates ACT/DVE.
                        qs = slice(j * QC, (j + 1) * QC)
                        for o in range(OCH):
                            ps = projp.tile([128, QC], F32, tag="proj")
                            for c in range(CCH):
                                nc.tensor.matmul(
                                    ps[:],
                                    wo_sb[:, c, o * 128:(o + 1) * 128],
                                    ctxn_pair[c][:, qs],
                                    start=(c == 0),
                                    stop=(c == CCH - 1),
                                )
                            # while interleaved into late attention, ACT is
                            # exp-critical: keep copies off it (DVE + Pool
                            # DMA); the final round alternates ACT/DVE
                            ot = outs.tile([128, QC], BF16, tag="ot")
                            if last and o % 2 == 0:
                                nc.scalar.activation(ot[:], ps[:], Act.Identity)
                                nc.scalar.dma_start(
                                    outT[o * 128:(o + 1) * 128, qs], ot[:]
                                )
                            else:
                                nc.vector.tensor_copy(ot[:], ps[:])
                                nc.gpsimd.dma_start(
                                    outT[o * 128:(o + 1) * 128, qs], ot[:]
                                )

                    qk_round(0)
                    es00 = scores_exp(0, 0)
                    es10 = scores_exp(1, 0)
                    qk_round(1)
                    es20 = scores_exp(2, 0)
                    es30 = scores_exp(3, 0)
                    v_round(0)
                    qk_round(2)
                    ctx_acc(0, 0, es00)
                    ctx_acc(1, 0, es10)
                    norm(0, 0)
                    es01 = scores_exp(0, 1)
                    v_round(1)
                    qk_round(3)
                    ctx_acc(2, 0, es20)
                    ctx_acc(3, 0, es30)
                    norm(1, 0)
                    es11 = scores_exp(1, 1)
                    ctx_acc(0, 1, es01)
                    v_round(2)
                    es21 = scores_exp(2, 1)
                    ctx_acc(1, 1, es11)
                    norm(0, 1)
                    es31 = scores_exp(3, 1)
                    v_round(3)
                    ctx_acc(2, 1, es21)
                    # depth-2 pipeline into j=2 so ACT always has exp queued
                    es02 = scores_exp(0, 2)
                    ctx_acc(3, 1, es31)
                    norm(1, 1)
                    es12 = scores_exp(1, 2)
                    ctx_acc(0, 2, es02)
                    es22 = scores_exp(2, 2)
                    ctx_acc(1, 2, es12)
                    norm(0, 2)
                    es32 = scores_exp(3, 2)
                    ctx_acc(2, 2, es22)
                    es03 = scores_exp(0, 3)
                    ctx_acc(3, 2, es32)
                    norm(1, 2)
                    es13 = scores_exp(1, 3)
                    ctx_acc(0, 3, es03)
                    out_round(0)
                    es23 = scores_exp(2, 3)
                    ctx_acc(1, 3, es13)
                    norm(0, 3)
                    out_round(1)
                    es33 = scores_exp(3, 3)
                    ctx_acc(2, 3, es23)
                    out_round(2)
                    ctx_acc(3, 3, es33)
                    norm(1, 3)
                    out_round(3, last=True)

            for _rep in range(reps):
                _emit_once()
    nc.compile()
    return nc


def _causal_mask():
    # msk[kp, r, qf] = 1 where (r*128 + kp) <= qf else 0  (keep k <= q)
    kp = np.arange(128)[:, None, None]
    r = np.arange(KPQ)[None, :, None]
    qf = np.arange(QC)[None, None, :]
    return (r * 128 + kp <= qf).astype(ml_dtypes.bfloat16)


def _stage(a, dtype, pdim=128):
    """[pdim*n, free...] -> contiguous [pdim, n, free...]"""
    n = a.shape[0] // pdim
    out = a.reshape((n, pdim) + a.shape[1:]).transpose(
        (1, 0) + tuple(range(2, a.ndim + 1))
    )
    return np.ascontiguousarray(out.astype(dtype))


def _in_maps(x, Wq, Wk, Wv, Wo, bo):
    bf = ml_dtypes.bfloat16
    f8 = ml_dtypes.float8_e4m3
    msk = _causal_mask()
    sel33 = np.zeros((33, 128), dtype=bf)
    sel33[0, 0:64] = 1.0
    sel33[32, 64:128] = 1.0
    xTs = [np.ascontiguousarray(x[b].T) for b in range(B)]
    x8s = [_stage(xb * X_SCALE, f8) for xb in xTs]
    xbs = [_stage(xb, bf) for xb in xTs]
    maps = []
    for c in range(NCORES):
        b, g = c // GROUP, c % GROUP
        cs = slice(g * CW, (g + 1) * CW)
        maps.append({
            "x8": x8s[b],
            "xT": xbs[b],
            "wq": _stage(Wq[:, cs] * W_SCALE, f8),
            "wk": _stage(Wk[:, cs] * W_SCALE, f8),
            "wv": _stage(Wv[:, cs], bf),
            "wo": _stage(np.ascontiguousarray(Wo[cs, :]), bf),
            "msk": msk,
            "vones": np.ones((128, NKC, HPC, 1), dtype=bf),
            "sel33": sel33,
        })
    return maps


def kernel(x, Wq, Wk, Wv, Wo, bo, _trace=False):
    x = np.asarray(x, dtype=np.float32)
    Wq, Wk, Wv, Wo, bo = (np.asarray(a, dtype=np.float32) for a in (Wq, Wk, Wv, Wo, bo))
    if "nc" not in _CACHE:
        _CACHE["nc"] = _build_bass()
    nc = _CACHE["nc"]
    res = run_bass_kernel_spmd(
        nc, _in_maps(x, Wq, Wk, Wv, Wo, bo), list(range(NCORES)), trace=_trace
    )
    out = np.zeros((B, S, D), dtype=np.float32)
    for b in range(B):
        acc = np.zeros((D, S), dtype=np.float32)
        for g in range(GROUP):
            acc += res.results[GROUP * b + g]["outT"].astype(np.float32)
        out[b] = acc.T + bo[None, :]
    if _trace:
        return out, res
    return out


# revision 25
# speedup vs baseline: 1.0207x; 1.0207x over previous
"""Distributed causal multi-head attention for Trainium2 (8 NeuronCores).

Problem (hardcoded): x[2, 2048, 1024], 16 heads, head_dim 64, causal
softmax(QK^T/8)V then out-proj with bias. f32 in/out.

Sharding: data parallel on batch (cores 0-3 -> batch 0, 4-7 -> batch 1),
tensor parallel on heads within each group of 4 (4 heads per core).
Each core:
  - computes Q^T,K^T via fp8(e4m3) DoubleRow matmuls (x pre-scaled by 8,
    Wq/Wk by 64 on the host; the 512^2 product scale is folded into the
    softmax exp scale), V in bf16
  - scores transposed S^T[k,q] = K Q^T so the softmax denominator comes out
    of the PE via an appended ones-column on V (no partition reductions)
  - exp without max-subtraction (scores are O(2), safe in fp32/bf16)
  - causal mask applied post-exp as a 0/1 bf16 multiply (DVE 2x mode)
  - ctx^T accumulated per q-chunk, normalized with 1/den partition-broadcast
  - row-parallel out-proj: each core computes the FULL-width partial
    outT_part[oc, q] = Wo[own 256 rows, oc]^T ctxT_own
No collectives: the host sums the 4 partial outputs per batch group
(standard row-parallel TP unshard), adds the bias, and transposes.

Attention/out-proj matmuls bf16 (fp32 PSUM accumulation); QK projections
fp8; partial outputs written bf16.
"""

import numpy as np
import ml_dtypes

from concourse import bass, bacc, mybir
from concourse import tile
from concourse.bass_utils import run_bass_kernel_spmd

BF16 = mybir.dt.bfloat16
F32 = mybir.dt.float32
FP8 = mybir.dt.float8e4
Act = mybir.ActivationFunctionType
DR = mybir.MatmulPerfMode.DoubleRow

B, S, D = 2, 2048, 1024
H, HD = 16, 64
NCORES = 8
GROUP = 4            # cores per batch group
HPC = H // GROUP     # 4 heads per core
CW = HPC * HD        # 256 ctx columns per core
QC = 512             # q-chunk width
KC = 128             # k-chunk width
NQ = S // QC         # 4
NKC = S // KC        # 16
KPQ = QC // KC       # 4 k-chunks per q-chunk
DCH = D // 128       # 8 contraction chunks of 128
CCH = CW // 128      # 2 own-ctx contraction chunks
OCH = D // 128       # 8 out-column chunks

X_SCALE = 8.0        # host pre-scale of x before fp8 quantization
W_SCALE = 64.0       # host pre-scale of Wq/Wk before fp8 quantization
# scores' = (512 q)·(512 k); exp(q·k/8) -> scale out the 512^2
EXP_SCALE = 0.125 / (X_SCALE * W_SCALE) ** 2

_CACHE = {}


def _build_bass(reps=1):
    nc = bacc.Bacc(
        "TRN2", target_bir_lowering=False, debug=False, num_devices=NCORES
    )

    # per-core external inputs, pre-staged by the host in SBUF layout
    # [partition, chunk, free] so each is a single large-descriptor DMA
    x8 = nc.declare_dram_parameter("x8", [128, DCH, S], FP8, isOutput=False)
    xT = nc.declare_dram_parameter("xT", [128, DCH, S], BF16, isOutput=False)
    wq = nc.declare_dram_parameter("wq", [128, DCH, CW], FP8, isOutput=False)
    wk = nc.declare_dram_parameter("wk", [128, DCH, CW], FP8, isOutput=False)
    wv = nc.declare_dram_parameter("wv", [128, DCH, CW], BF16, isOutput=False)
    wo = nc.declare_dram_parameter("wo", [128, CCH, D], BF16, isOutput=False)
    msk = nc.declare_dram_parameter("msk", [128, KPQ, QC], BF16, isOutput=False)
    vones = nc.declare_dram_parameter("vones", [128, NKC, HPC, 1], BF16, isOutput=False)
    # selector for den broadcast: bc[m,q] = sum_k sel33[k,m]*den_pair[k,q]
    sel33 = nc.declare_dram_parameter("sel33", [33, 128], BF16, isOutput=False)
    outT = nc.declare_dram_parameter("outT", [D, S], BF16, isOutput=True)

    with tile.TileContext(nc) as tc:
        with tc.tile_pool(name="persist", bufs=1) as pp:
            x8_sb = pp.tile([128, DCH, S], FP8, tag="x8_sb")
            xT_sb = pp.tile([128, DCH, S], BF16, tag="xT_sb")
            wq_sb = pp.tile([128, DCH, CW], FP8, tag="wq_sb")
            wk_sb = pp.tile([128, DCH, CW], FP8, tag="wk_sb")
            wv_sb = pp.tile([128, DCH, CW], BF16, tag="wv_sb")
            wo_sb = pp.tile([128, CCH, D], BF16, tag="wo_sb")
            msk_sb = pp.tile([128, KPQ, QC], BF16, tag="msk_sb")
            qT_sb = pp.tile([128, 2, S], BF16, tag="qT_sb")
            kT_sb = pp.tile([128, 2, S], BF16, tag="kT_sb")
            v_aug = pp.tile([128, NKC, HPC, HD + 1], BF16, tag="v_aug")
            ctxu0 = pp.tile([128, S], F32, tag="ctxu0")
            ctxu1 = pp.tile([128, S], F32, tag="ctxu1")
            ctxn0 = pp.tile([128, S], BF16, tag="ctxn0")
            ctxn1 = pp.tile([128, S], BF16, tag="ctxn1")
            # den per pair: head 2p at partition 0, head 2p+1 at partition
            # 32 (ACT writes must start at multiples of 32); rows 1-31 are
            # zeroed so the K=33 selector matmul can broadcast both heads
            # to output partitions 0-63 / 64-127 in one instruction
            den_pair = [pp.tile([33, S], BF16, tag=f"den{p}", name=f"den{p}")
                        for p in range(2)]
            sel_sb = pp.tile([33, 128], BF16, tag="sel_sb")
            ctxu_pair = [ctxu0, ctxu1]
            ctxn_pair = [ctxn0, ctxn1]
            # Pool engine (idle otherwise) zeroes the den scratch
            for p in range(2):
                nc.gpsimd.memset(den_pair[p][:], 0.0)

            # DMA order = first-use order. The first x8 window is split
            # per d-chunk so the very first projection chain unblocks
            # after wq + one small chunk; everything else streams behind.
            # All of x8 (fp8, 2MB) lands before xT (bf16, 4MB) so the QK
            # projections and all j=0 scores can run while V streams in.
            def _x8w(w):
                nc.sync.dma_start(
                    x8_sb[:, :, w * QC:(w + 1) * QC],
                    x8[:, :, w * QC:(w + 1) * QC],
                )

            def _xTw(w):
                nc.sync.dma_start(
                    xT_sb[:, :, w * QC:(w + 1) * QC],
                    xT[:, :, w * QC:(w + 1) * QC],
                )

            nc.sync.dma_start(wq_sb[:], wq[:])
            nc.sync.dma_start(wk_sb[:], wk[:])
            for c in range(DCH):
                nc.sync.dma_start(x8_sb[:, c, 0:QC], x8[:, c, 0:QC])
            nc.sync.dma_start(msk_sb[:], msk[:])
            for w in range(1, NQ):
                _x8w(w)
            nc.sync.dma_start(wv_sb[:], wv[:])
            # ones column of V_aug comes from the host: keeps the V
            # PSUM->SBUF copy to a single (PE) sync wait
            nc.sync.dma_start(v_aug[:, :, :, HD:HD + 1], vones[:])
            nc.sync.dma_start(sel_sb[:], sel33[:])
            for w in range(NQ):
                _xTw(w)
            nc.sync.dma_start(wo_sb[:], wo[:])

            def _emit_once():
                with tc.tile_pool(name="proj_ps", bufs=2, space="PSUM") as projp, \
                     tc.tile_pool(name="sc_ps", bufs=2, space="PSUM") as scp, \
                     tc.tile_pool(name="ctbc_ps", bufs=2, space="PSUM") as ctp, \
                     tc.tile_pool(name="es_pool", bufs=22) as esp, \
                     tc.tile_pool(name="out_sb", bufs=8) as outs, \
                     tc.tile_pool(name="norm", bufs=2) as np_pool:

                    def qk_round(j):
                        # Q,K projections for q/k-token window j, both
                        # pairs; fp8 DoubleRow: chunk pairs -> K=256
                        for pair in range(2):
                            for w_sb, dst in ((wq_sb, qT_sb), (wk_sb, kT_sb)):
                                ps = projp.tile([128, QC], F32, tag="proj")
                                for c in range(0, DCH, 2):
                                    nc.tensor.matmul(
                                        ps[:],
                                        w_sb[:, c:c + 2, pair * 128:(pair + 1) * 128],
                                        x8_sb[:, c:c + 2, j * QC:(j + 1) * QC],
                                        start=(c == 0),
                                        stop=(c == DCH - 2),
                                        perf_mode=DR,
                                    )
                                nc.vector.tensor_copy(
                                    dst[:, pair, j * QC:(j + 1) * QC], ps[:]
                                )

                    def v_round(w):
                        # V for token chunks 4w..4w+3, both pairs
                        for t in range(4 * w, 4 * w + 4):
                            for pair in range(2):
                                ps = projp.tile([128, QC], F32, tag="proj")
                                for c in range(DCH):
                                    nc.tensor.matmul(
                                        ps[:, 0:128],
                                        xT_sb[:, c, t * 128:(t + 1) * 128],
                                        wv_sb[:, c, pair * 128:(pair + 1) * 128],
                                        start=(c == 0),
                                        stop=(c == DCH - 1),
                                    )
                                nc.vector.tensor_copy(
                                    v_aug[:, t, 2 * pair:2 * pair + 2, 0:HD],
                                    ps[:, 0:128].rearrange("p (h w) -> p h w", h=2),
                                )

                    def scores_exp(h, j):
                        """S^T then exp (+ causal masking) for q-chunk j of
                        head h. Off-band k-chunks (fully below the diagonal)
                        get full-width matmuls; the 4-chunk diagonal band
                        uses shrinking q-windows (exact block causality)
                        with a 128-wide triangle mask per chunk. Returns the
                        es tiles (off-band pairs + band tiles 1 and 2)."""
                        pair, hh = h // 2, h % 2
                        row = hh * 64
                        qs = slice(j * QC, (j + 1) * QC)
                        es_tiles = []
                        # off-band: k-chunks 0 .. 4j-1, two per PSUM tile
                        for c0 in range(0, 4 * j, 2):
                            st = scp.tile([128, 2, QC], F32, tag="st")
                            for i in range(2):
                                c = c0 + i
                                nc.tensor.matmul(
                                    st[:, i, :],
                                    kT_sb[row:row + 64, pair, c * KC:(c + 1) * KC],
                                    qT_sb[row:row + 64, pair, qs],
                                    start=True, stop=True,
                                )
                            es = esp.tile([128, 2, QC], BF16, tag="es")
                            nc.scalar.activation(es[:], st[:], Act.Exp,
                                                 scale=EXP_SCALE)
                            es_tiles.append(es)
                        # diagonal band: k-chunks 4j+r, q-window [128r, 512)
                        # packed as two tiles; sub-window starts snapped so
                        # each exp is one full-AP instruction (the unwritten
                        # PSUM slivers are exp'd but never read)
                        win = [0, KC, 2 * KC, 3 * KC]
                        for ti in range(2):
                            st = scp.tile([128, 2, QC], F32, tag="st")
                            for i in range(2):
                                r = 2 * ti + i
                                w0 = win[r]
                                nc.tensor.matmul(
                                    st[:, i, w0:QC],
                                    kT_sb[row:row + 64, pair,
                                          (4 * j + r) * KC:(4 * j + r + 1) * KC],
                                    qT_sb[row:row + 64, pair,
                                          j * QC + w0:(j + 1) * QC],
                                    start=True, stop=True,
                                )
                            es = esp.tile([128, 2, QC], BF16, tag="es")
                            lo = win[2 * ti]
                            nc.scalar.activation(es[:, :, lo:QC],
                                                 st[:, :, lo:QC],
                                                 Act.Exp, scale=EXP_SCALE)
                            # triangle mask on the leading 128 q of each
                            # chunk's window
                            for i in range(2):
                                r = 2 * ti + i
                                w0 = win[r]
                                nc.vector.tensor_mul(
                                    es[:, i, w0:w0 + KC], es[:, i, w0:w0 + KC],
                                    msk_sb[:, r, w0:w0 + KC],
                                )
                            es_tiles.append(es)
                        return es_tiles

                    def ctx_acc(h, j, es_tiles):
                        """attn@V accumulation + write-back for (h, j)."""
                        pair, hh = h // 2, h % 2
                        row = hh * 64
                        qs = slice(j * QC, (j + 1) * QC)
                        win = [0, KC, 2 * KC, 3 * KC]
                        ct = ctp.tile([HD + 1, QC], F32, tag="ct")
                        for c in range(4 * j):
                            nc.tensor.matmul(
                                ct[:],
                                v_aug[:, c, h, :],
                                es_tiles[c // 2][:, c % 2, :],
                                start=(c == 0),
                                stop=False,
                            )
                        for r in range(4):
                            w0 = win[r]
                            nc.tensor.matmul(
                                ct[:, w0:QC],
                                v_aug[:, 4 * j + r, h, :],
                                es_tiles[2 * j + r // 2][:, r % 2, w0:QC],
                                start=(j == 0 and r == 0),
                                stop=(r == 3),
                            )
                        nc.vector.tensor_copy(
                            ctxu_pair[pair][row:row + 64, qs], ct[0:HD, :]
                        )
                        nc.vector.tensor_copy(
                            den_pair[pair][hh * 32:hh * 32 + 1, qs],
                            ct[HD:HD + 1, :],
                        )

                    def norm(pair, j):
                        # den[pair][:, qs] complete once both heads of the
                        # pair finished ctx for q-chunk j
                        qs = slice(j * QC, (j + 1) * QC)
                        bc = ctp.tile([128, QC], F32, tag="ct")
                        nc.tensor.matmul(
                            bc[:], sel_sb[:], den_pair[pair][:, qs],
                            start=True, stop=True,
                        )
                        rb = np_pool.tile([128, QC], F32, tag="rb")
                        nc.vector.reciprocal(rb[:], bc[:])
                        nc.vector.tensor_mul(
                            ctxn_pair[pair][:, qs],
                            ctxu_pair[pair][:, qs], rb[:],
                        )

                    # Interleaved emission. QK projections and all j=0
                    # scores depend only on x8 (fp8, lands first), so they
                    # run while the larger bf16 x for V is still streaming;
                    # exp starts ~15us earlier than a phase-ordered kernel.
                    # The attention pipeline emits scores of the next chunk
                    # before ctx of the previous one so the PE has queued
                    # work while ACT runs exp; each (pair, j) normalizes as
                    # soon as its den is complete.
                    def out_round(j, last=False):
                        # row-parallel out-proj for q-chunk j:
                        # outT_part[oc, q] = Wo[own, oc]^T ctxn_own (bias on
                        # host). PSUM alternates between the projection pool
                        # (dead once the v_rounds finish) and a scores-pool
                        # slice so the matmuls can run ~2 copies ahead.
                        # While interleaved into late attention (ACT is
                        # exp-critical there) all copies go to DVE with the
                        # DMA issued from the idle Pool queue; the final
                        # round alternates ACT/DVE.
                        qs = slice(j * QC, (j + 1) * QC)
                        for o in range(OCH):
                            ps = projp.tile([128, QC], F32, tag="proj")
                            for c in range(CCH):
                                nc.tensor.matmul(
                                    ps[:],
                                    wo_sb[:, c, o * 128:(o + 1) * 128],
                                    ctxn_pair[c][:, qs],
                                    start=(c == 0),
                                    stop=(c == CCH - 1),
                                )
                            # while interleaved into late attention, ACT is
                            # exp-critical: keep copies off it (DVE + Pool
                            # DMA); the final round alternates ACT/DVE
                            ot = outs.tile([128, QC], BF16, tag="ot")
                            if last and o % 2 == 0:
                                nc.scalar.activation(ot[:], ps[:], Act.Identity)
                                nc.scalar.dma_start(
                                    outT[o * 128:(o + 1) * 128, qs], ot[:]
                                )
                            else:
                                nc.vector.tensor_copy(ot[:], ps[:])
                                nc.gpsimd.dma_start(
                                    outT[o * 128:(o + 1) * 128, qs], ot[:]
                                )

                    qk_round(0)
                    es00 = scores_exp(0, 0)
                    es10 = scores_exp(1, 0)
                    qk_round(1)
                    es20 = scores_exp(2, 0)
                    es30 = scores_exp(3, 0)
                    qk_round(2)
                    qk_round(3)
                    v_round(0)
                    ctx_acc(0, 0, es00)
                    ctx_acc(1, 0, es10)
                    norm(0, 0)
                    es01 = scores_exp(0, 1)
                    v_round(1)
                    ctx_acc(2, 0, es20)
                    ctx_acc(3, 0, es30)
                    norm(1, 0)
                    es11 = scores_exp(1, 1)
                    ctx_acc(0, 1, es01)
                    v_round(2)
                    es21 = scores_exp(2, 1)
                    ctx_acc(1, 1, es11)
                    norm(0, 1)
                    v_round(3)
                    es31 = scores_exp(3, 1)
                    ctx_acc(2, 1, es21)
                    # depth-2 pipeline into j=2 so ACT always has exp queued
                    es02 = scores_exp(0, 2)
                    ctx_acc(3, 1, es31)
                    norm(1, 1)
                    es12 = scores_exp(1, 2)
                    ctx_acc(0, 2, es02)
                    es22 = scores_exp(2, 2)
                    ctx_acc(1, 2, es12)
                    norm(0, 2)
                    es32 = scores_exp(3, 2)
                    ctx_acc(2, 2, es22)
                    es03 = scores_exp(0, 3)
                    ctx_acc(3, 2, es32)
                    norm(1, 2)
                    es13 = scores_exp(1, 3)
                    ctx_acc(0, 3, es03)
                    out_round(0)
                    es23 = scores_exp(2, 3)
                    ctx_acc(1, 3, es13)
                    norm(0, 3)
                    out_round(1)
                    es33 = scores_exp(3, 3)
                    ctx_acc(2, 3, es23)
                    out_round(2)
                    ctx_acc(3, 3, es33)
                    norm(1, 3)
                    out_round(3, last=True)

            for _rep in range(reps):
                _emit_once()
    nc.compile()
    return nc


def _causal_mask():
    # msk[kp, r, qf] = 1 where (r*128 + kp) <= qf else 0  (keep k <= q)
    kp = np.arange(128)[:, None, None]
    r = np.arange(KPQ)[None, :, None]
    qf = np.arange(QC)[None, None, :]
    return (r * 128 + kp <= qf).astype(ml_dtypes.bfloat16)


def _stage(a, dtype, pdim=128):
    """[pdim*n, free...] -> contiguous [pdim, n, free...]"""
    n = a.shape[0] // pdim
    out = a.reshape((n, pdim) + a.shape[1:]).transpose(
        (1, 0) + tuple(range(2, a.ndim + 1))
    )
    return np.ascontiguousarray(out.astype(dtype))


def _in_maps(x, Wq, Wk, Wv, Wo, bo):
    bf = ml_dtypes.bfloat16
    f8 = ml_dtypes.float8_e4m3
    msk = _causal_mask()
    sel33 = np.zeros((33, 128), dtype=bf)
    sel33[0, 0:64] = 1.0
    sel33[32, 64:128] = 1.0
    xTs = [np.ascontiguousarray(x[b].T) for b in range(B)]
    x8s = [_stage(xb * X_SCALE, f8) for xb in xTs]
    xbs = [_stage(xb, bf) for xb in xTs]
    maps = []
    for c in range(NCORES):
        b, g = c // GROUP, c % GROUP
        cs = slice(g * CW, (g + 1) * CW)
        maps.append({
            "x8": x8s[b],
            "xT": xbs[b],
            "wq": _stage(Wq[:, cs] * W_SCALE, f8),
            "wk": _stage(Wk[:, cs] * W_SCALE, f8),
            "wv": _stage(Wv[:, cs], bf),
            "wo": _stage(np.ascontiguousarray(Wo[cs, :]), bf),
            "msk": msk,
            "vones": np.ones((128, NKC, HPC, 1), dtype=bf),
            "sel33": sel33,
        })
    return maps


def kernel(x, Wq, Wk, Wv, Wo, bo, _trace=False):
    x = np.asarray(x, dtype=np.float32)
    Wq, Wk, Wv, Wo, bo = (np.asarray(a, dtype=np.float32) for a in (Wq, Wk, Wv, Wo, bo))
    if "nc" not in _CACHE:
        _CACHE["nc"] = _build_bass()
    nc = _CACHE["nc"]
    res = run_bass_kernel_spmd(
        nc, _in_maps(x, Wq, Wk, Wv, Wo, bo), list(range(NCORES)), trace=_trace
    )
    out = np.zeros((B, S, D), dtype=np.float32)
    for b in range(B):
        acc = np.zeros((D, S), dtype=np.float32)
        for g in range(GROUP):
            acc += res.results[GROUP * b + g]["outT"].astype(np.float32)
        out[b] = acc.T + bo[None, :]
    if _trace:
        return out, res
    return out


# revision 27
# speedup vs baseline: 1.0214x; 1.0006x over previous
"""Distributed causal multi-head attention for Trainium2 (8 NeuronCores).

Problem (hardcoded): x[2, 2048, 1024], 16 heads, head_dim 64, causal
softmax(QK^T/8)V then out-proj with bias. f32 in/out.

Sharding: data parallel on batch (cores 0-3 -> batch 0, 4-7 -> batch 1),
tensor parallel on heads within each group of 4 (4 heads per core).
Each core:
  - computes Q^T,K^T via fp8(e4m3) DoubleRow matmuls (x pre-scaled by 8,
    Wq/Wk by 64 on the host; the 512^2 product scale is folded into the
    softmax exp scale), V in bf16
  - scores transposed S^T[k,q] = K Q^T so the softmax denominator comes out
    of the PE via an appended ones-column on V (no partition reductions)
  - exp without max-subtraction (scores are O(2), safe in fp32/bf16)
  - causal mask applied post-exp as a 0/1 bf16 multiply (DVE 2x mode)
  - ctx^T accumulated per q-chunk, normalized with 1/den partition-broadcast
  - row-parallel out-proj: each core computes the FULL-width partial
    outT_part[oc, q] = Wo[own 256 rows, oc]^T ctxT_own
No collectives: the host sums the 4 partial outputs per batch group
(standard row-parallel TP unshard), adds the bias, and transposes.

Attention/out-proj matmuls bf16 (fp32 PSUM accumulation); QK projections
fp8; partial outputs written bf16.
"""

import numpy as np
import ml_dtypes

from concourse import bass, bacc, mybir
from concourse import tile
from concourse.bass_utils import run_bass_kernel_spmd

BF16 = mybir.dt.bfloat16
F32 = mybir.dt.float32
FP8 = mybir.dt.float8e4
Act = mybir.ActivationFunctionType
DR = mybir.MatmulPerfMode.DoubleRow

B, S, D = 2, 2048, 1024
H, HD = 16, 64
NCORES = 8
GROUP = 4            # cores per batch group
HPC = H // GROUP     # 4 heads per core
CW = HPC * HD        # 256 ctx columns per core
QC = 512             # q-chunk width
KC = 128             # k-chunk width
NQ = S // QC         # 4
NKC = S // KC        # 16
KPQ = QC // KC       # 4 k-chunks per q-chunk
DCH = D // 128       # 8 contraction chunks of 128
CCH = CW // 128      # 2 own-ctx contraction chunks
OCH = D // 128       # 8 out-column chunks

X_SCALE = 8.0        # host pre-scale of x before fp8 quantization
W_SCALE = 64.0       # host pre-scale of Wq/Wk before fp8 quantization
# scores' = (512 q)·(512 k); exp(q·k/8) -> scale out the 512^2
EXP_SCALE = 0.125 / (X_SCALE * W_SCALE) ** 2

_CACHE = {}


def _build_bass(reps=1):
    nc = bacc.Bacc(
        "TRN2", target_bir_lowering=False, debug=False, num_devices=NCORES
    )

    # per-core external inputs, pre-staged by the host in SBUF layout
    # [partition, chunk, free] so each is a single large-descriptor DMA
    x8 = nc.declare_dram_parameter("x8", [128, DCH, S], FP8, isOutput=False)
    xT = nc.declare_dram_parameter("xT", [128, DCH, S], BF16, isOutput=False)
    wq = nc.declare_dram_parameter("wq", [128, DCH, CW], FP8, isOutput=False)
    wk = nc.declare_dram_parameter("wk", [128, DCH, CW], FP8, isOutput=False)
    wv = nc.declare_dram_parameter("wv", [128, DCH, CW], BF16, isOutput=False)
    wo = nc.declare_dram_parameter("wo", [128, CCH, D], BF16, isOutput=False)
    msk = nc.declare_dram_parameter("msk", [128, KPQ, QC], BF16, isOutput=False)
    vones = nc.declare_dram_parameter("vones", [128, NKC, HPC, 1], BF16, isOutput=False)
    # selector for den broadcast: bc[m,q] = sum_k sel33[k,m]*den_pair[k,q]
    sel33 = nc.declare_dram_parameter("sel33", [33, 128], BF16, isOutput=False)
    outT = nc.declare_dram_parameter("outT", [D, S], BF16, isOutput=True)

    with tile.TileContext(nc) as tc:
        with tc.tile_pool(name="persist", bufs=1) as pp:
            x8_sb = pp.tile([128, DCH, S], FP8, tag="x8_sb")
            xT_sb = pp.tile([128, DCH, S], BF16, tag="xT_sb")
            wq_sb = pp.tile([128, DCH, CW], FP8, tag="wq_sb")
            wk_sb = pp.tile([128, DCH, CW], FP8, tag="wk_sb")
            wv_sb = pp.tile([128, DCH, CW], BF16, tag="wv_sb")
            wo_sb = pp.tile([128, CCH, D], BF16, tag="wo_sb")
            msk_sb = pp.tile([128, KPQ, QC], BF16, tag="msk_sb")
            qT_sb = pp.tile([128, 2, S], BF16, tag="qT_sb")
            kT_sb = pp.tile([128, 2, S], BF16, tag="kT_sb")
            v_aug = pp.tile([128, NKC, HPC, HD + 1], BF16, tag="v_aug")
            ctxu0 = pp.tile([128, S], F32, tag="ctxu0")
            ctxu1 = pp.tile([128, S], F32, tag="ctxu1")
            ctxn0 = pp.tile([128, S], BF16, tag="ctxn0")
            ctxn1 = pp.tile([128, S], BF16, tag="ctxn1")
            # den per pair: head 2p at partition 0, head 2p+1 at partition
            # 32 (ACT writes must start at multiples of 32); rows 1-31 are
            # zeroed so the K=33 selector matmul can broadcast both heads
            # to output partitions 0-63 / 64-127 in one instruction
            den_pair = [pp.tile([33, S], BF16, tag=f"den{p}", name=f"den{p}")
                        for p in range(2)]
            sel_sb = pp.tile([33, 128], BF16, tag="sel_sb")
            ctxu_pair = [ctxu0, ctxu1]
            ctxn_pair = [ctxn0, ctxn1]
            # Pool engine (idle otherwise) zeroes the den scratch
            for p in range(2):
                nc.gpsimd.memset(den_pair[p][:], 0.0)

            # DMA order = first-use order. The first x8 window is split
            # per d-chunk so the very first projection chain unblocks
            # after wq + one small chunk; everything else streams behind.
            # All of x8 (fp8, 2MB) lands before xT (bf16, 4MB) so the QK
            # projections and all j=0 scores can run while V streams in.
            def _x8w(w):
                nc.sync.dma_start(
                    x8_sb[:, :, w * QC:(w + 1) * QC],
                    x8[:, :, w * QC:(w + 1) * QC],
                )

            def _xTw(w):
                nc.sync.dma_start(
                    xT_sb[:, :, w * QC:(w + 1) * QC],
                    xT[:, :, w * QC:(w + 1) * QC],
                )

            nc.sync.dma_start(wq_sb[:], wq[:])
            nc.sync.dma_start(wk_sb[:], wk[:])
            for c in range(DCH):
                nc.sync.dma_start(x8_sb[:, c, 0:QC], x8[:, c, 0:QC])
            nc.sync.dma_start(msk_sb[:], msk[:])
            for w in range(1, NQ):
                _x8w(w)
            nc.sync.dma_start(wv_sb[:], wv[:])
            # ones column of V_aug comes from the host: keeps the V
            # PSUM->SBUF copy to a single (PE) sync wait
            nc.sync.dma_start(v_aug[:, :, :, HD:HD + 1], vones[:])
            nc.sync.dma_start(sel_sb[:], sel33[:])
            for w in range(NQ):
                _xTw(w)
            nc.sync.dma_start(wo_sb[:], wo[:])

            def _emit_once():
                with tc.tile_pool(name="proj_ps", bufs=2, space="PSUM") as projp, \
                     tc.tile_pool(name="sc_ps", bufs=2, space="PSUM") as scp, \
                     tc.tile_pool(name="ctbc_ps", bufs=2, space="PSUM") as ctp, \
                     tc.tile_pool(name="es_pool", bufs=26) as esp, \
                     tc.tile_pool(name="out_sb", bufs=8) as outs, \
                     tc.tile_pool(name="norm", bufs=2) as np_pool:

                    def qk_round(j):
                        # Q,K projections for q/k-token window j, both
                        # pairs; fp8 DoubleRow: chunk pairs -> K=256
                        for pair in range(2):
                            for w_sb, dst in ((wq_sb, qT_sb), (wk_sb, kT_sb)):
                                ps = projp.tile([128, QC], F32, tag="proj")
                                for c in range(0, DCH, 2):
                                    nc.tensor.matmul(
                                        ps[:],
                                        w_sb[:, c:c + 2, pair * 128:(pair + 1) * 128],
                                        x8_sb[:, c:c + 2, j * QC:(j + 1) * QC],
                                        start=(c == 0),
                                        stop=(c == DCH - 2),
                                        perf_mode=DR,
                                    )
                                nc.vector.tensor_copy(
                                    dst[:, pair, j * QC:(j + 1) * QC], ps[:]
                                )

                    def v_round(w):
                        # V for token chunks 4w..4w+3, both pairs
                        for t in range(4 * w, 4 * w + 4):
                            for pair in range(2):
                                ps = projp.tile([128, QC], F32, tag="proj")
                                for c in range(DCH):
                                    nc.tensor.matmul(
                                        ps[:, 0:128],
                                        xT_sb[:, c, t * 128:(t + 1) * 128],
                                        wv_sb[:, c, pair * 128:(pair + 1) * 128],
                                        start=(c == 0),
                                        stop=(c == DCH - 1),
                                    )
                                nc.vector.tensor_copy(
                                    v_aug[:, t, 2 * pair:2 * pair + 2, 0:HD],
                                    ps[:, 0:128].rearrange("p (h w) -> p h w", h=2),
                                )

                    def scores_exp(h, j):
                        """S^T then exp (+ causal masking) for q-chunk j of
                        head h. Off-band k-chunks (fully below the diagonal)
                        get full-width matmuls; the 4-chunk diagonal band
                        uses shrinking q-windows (exact block causality)
                        with a 128-wide triangle mask per chunk. Returns the
                        es tiles (off-band pairs + band tiles 1 and 2)."""
                        pair, hh = h // 2, h % 2
                        row = hh * 64
                        qs = slice(j * QC, (j + 1) * QC)
                        es_tiles = []
                        # off-band: k-chunks 0 .. 4j-1, two per PSUM tile
                        for c0 in range(0, 4 * j, 2):
                            st = scp.tile([128, 2, QC], F32, tag="st")
                            for i in range(2):
                                c = c0 + i
                                nc.tensor.matmul(
                                    st[:, i, :],
                                    kT_sb[row:row + 64, pair, c * KC:(c + 1) * KC],
                                    qT_sb[row:row + 64, pair, qs],
                                    start=True, stop=True,
                                )
                            es = esp.tile([128, 2, QC], BF16, tag="es")
                            nc.scalar.activation(es[:], st[:], Act.Exp,
                                                 scale=EXP_SCALE)
                            es_tiles.append(es)
                        # diagonal band: k-chunks 4j+r, q-window [128r, 512)
                        # packed as two tiles; sub-window starts snapped so
                        # each exp is one full-AP instruction (the unwritten
                        # PSUM slivers are exp'd but never read)
                        win = [0, KC, 2 * KC, 3 * KC]
                        for ti in range(2):
                            st = scp.tile([128, 2, QC], F32, tag="st")
                            for i in range(2):
                                r = 2 * ti + i
                                w0 = win[r]
                                nc.tensor.matmul(
                                    st[:, i, w0:QC],
                                    kT_sb[row:row + 64, pair,
                                          (4 * j + r) * KC:(4 * j + r + 1) * KC],
                                    qT_sb[row:row + 64, pair,
                                          j * QC + w0:(j + 1) * QC],
                                    start=True, stop=True,
                                )
                            es = esp.tile([128, 2, QC], BF16, tag="es")
                            lo = win[2 * ti]
                            nc.scalar.activation(es[:, :, lo:QC],
                                                 st[:, :, lo:QC],
                                                 Act.Exp, scale=EXP_SCALE)
                            # triangle mask on the leading 128 q of each
                            # chunk's window
                            for i in range(2):
                                r = 2 * ti + i
                                w0 = win[r]
                                nc.vector.tensor_mul(
                                    es[:, i, w0:w0 + KC], es[:, i, w0:w0 + KC],
                                    msk_sb[:, r, w0:w0 + KC],
                                )
                            es_tiles.append(es)
                        return es_tiles

                    def ctx_acc(h, j, es_tiles):
                        """attn@V accumulation + write-back for (h, j)."""
                        pair, hh = h // 2, h % 2
                        row = hh * 64
                        qs = slice(j * QC, (j + 1) * QC)
                        win = [0, KC, 2 * KC, 3 * KC]
                        ct = ctp.tile([HD + 1, QC], F32, tag="ct")
                        for c in range(4 * j):
                            nc.tensor.matmul(
                                ct[:],
                                v_aug[:, c, h, :],
                                es_tiles[c // 2][:, c % 2, :],
                                start=(c == 0),
                                stop=False,
                            )
                        for r in range(4):
                            w0 = win[r]
                            nc.tensor.matmul(
                                ct[:, w0:QC],
                                v_aug[:, 4 * j + r, h, :],
                                es_tiles[2 * j + r // 2][:, r % 2, w0:QC],
                                start=(j == 0 and r == 0),
                                stop=(r == 3),
                            )
                        nc.vector.tensor_copy(
                            ctxu_pair[pair][row:row + 64, qs], ct[0:HD, :]
                        )
                        nc.vector.tensor_copy(
                            den_pair[pair][hh * 32:hh * 32 + 1, qs],
                            ct[HD:HD + 1, :],
                        )

                    def norm(pair, j):
                        # den[pair][:, qs] complete once both heads of the
                        # pair finished ctx for q-chunk j
                        qs = slice(j * QC, (j + 1) * QC)
                        bc = ctp.tile([128, QC], F32, tag="ct")
                        nc.tensor.matmul(
                            bc[:], sel_sb[:], den_pair[pair][:, qs],
                            start=True, stop=True,
                        )
                        rb = np_pool.tile([128, QC], F32, tag="rb")
                        nc.vector.reciprocal(rb[:], bc[:])
                        nc.vector.tensor_mul(
                            ctxn_pair[pair][:, qs],
                            ctxu_pair[pair][:, qs], rb[:],
                        )

                    # Interleaved emission. QK projections and all j=0
                    # scores depend only on x8 (fp8, lands first), so they
                    # run while the larger bf16 x for V is still streaming;
                    # exp starts ~15us earlier than a phase-ordered kernel.
                    # The attention pipeline emits scores of the next chunk
                    # before ctx of the previous one so the PE has queued
                    # work while ACT runs exp; each (pair, j) normalizes as
                    # soon as its den is complete.
                    def out_round(j, last=False):
                        # row-parallel out-proj for q-chunk j:
                        # outT_part[oc, q] = Wo[own, oc]^T ctxn_own (bias on
                        # host). PSUM alternates between the projection pool
                        # (dead once the v_rounds finish) and a scores-pool
                        # slice so the matmuls can run ~2 copies ahead.
                        # While interleaved into late attention (ACT is
                        # exp-critical there) all copies go to DVE with the
                        # DMA issued from the idle Pool queue; the final
                        # round alternates ACT/DVE.
                        qs = slice(j * QC, (j + 1) * QC)
                        for o in range(OCH):
                            ps = projp.tile([128, QC], F32, tag="proj")
                            for c in range(CCH):
                                nc.tensor.matmul(
                                    ps[:],
                                    wo_sb[:, c, o * 128:(o + 1) * 128],
                                    ctxn_pair[c][:, qs],
                                    start=(c == 0),
                                    stop=(c == CCH - 1),
                                )
                            # while interleaved into late attention, ACT is
                            # exp-critical: keep copies off it (DVE + Pool
                            # DMA); the final round alternates ACT/DVE
                            ot = outs.tile([128, QC], BF16, tag="ot")
                            if last and o % 2 == 0:
                                nc.scalar.activation(ot[:], ps[:], Act.Identity)
                                nc.scalar.dma_start(
                                    outT[o * 128:(o + 1) * 128, qs], ot[:]
                                )
                            else:
                                nc.vector.tensor_copy(ot[:], ps[:])
                                nc.gpsimd.dma_start(
                                    outT[o * 128:(o + 1) * 128, qs], ot[:]
                                )

                    qk_round(0)
                    es00 = scores_exp(0, 0)
                    es10 = scores_exp(1, 0)
                    qk_round(1)
                    es20 = scores_exp(2, 0)
                    es30 = scores_exp(3, 0)
                    qk_round(2)
                    qk_round(3)
                    v_round(0)
                    ctx_acc(0, 0, es00)
                    ctx_acc(1, 0, es10)
                    norm(0, 0)
                    es01 = scores_exp(0, 1)
                    v_round(1)
                    ctx_acc(2, 0, es20)
                    ctx_acc(3, 0, es30)
                    norm(1, 0)
                    es11 = scores_exp(1, 1)
                    ctx_acc(0, 1, es01)
                    v_round(2)
                    es21 = scores_exp(2, 1)
                    ctx_acc(1, 1, es11)
                    norm(0, 1)
                    v_round(3)
                    es31 = scores_exp(3, 1)
                    ctx_acc(2, 1, es21)
                    # depth-2 pipeline into j=2 so ACT always has exp queued
                    es02 = scores_exp(0, 2)
                    ctx_acc(3, 1, es31)
                    norm(1, 1)
                    es12 = scores_exp(1, 2)
                    ctx_acc(0, 2, es02)
                    es22 = scores_exp(2, 2)
                    ctx_acc(1, 2, es12)
                    norm(0, 2)
                    es32 = scores_exp(3, 2)
                    ctx_acc(2, 2, es22)
                    es03 = scores_exp(0, 3)
                    ctx_acc(3, 2, es32)
                    norm(1, 2)
                    es13 = scores_exp(1, 3)
                    ctx_acc(0, 3, es03)
                    out_round(0)
                    es23 = scores_exp(2, 3)
                    ctx_acc(1, 3, es13)
                    norm(0, 3)
                    es33 = scores_exp(3, 3)
                    out_round(1)
                    ctx_acc(2, 3, es23)
                    out_round(2)
                    ctx_acc(3, 3, es33)
                    norm(1, 3)
                    out_round(3, last=True)

            for _rep in range(reps):
                _emit_once()
    nc.compile()
    return nc


def _causal_mask():
    # msk[kp, r, qf] = 1 where (r*128 + kp) <= qf else 0  (keep k <= q)
    kp = np.arange(128)[:, None, None]
    r = np.arange(KPQ)[None, :, None]
    qf = np.arange(QC)[None, None, :]
    return (r * 128 + kp <= qf).astype(ml_dtypes.bfloat16)


def _stage(a, dtype, pdim=128):
    """[pdim*n, free...] -> contiguous [pdim, n, free...]"""
    n = a.shape[0] // pdim
    out = a.reshape((n, pdim) + a.shape[1:]).transpose(
        (1, 0) + tuple(range(2, a.ndim + 1))
    )
    return np.ascontiguousarray(out.astype(dtype))


def _in_maps(x, Wq, Wk, Wv, Wo, bo):
    bf = ml_dtypes.bfloat16
    f8 = ml_dtypes.float8_e4m3
    msk = _causal_mask()
    sel33 = np.zeros((33, 128), dtype=bf)
    sel33[0, 0:64] = 1.0
    sel33[32, 64:128] = 1.0
    xTs = [np.ascontiguousarray(x[b].T) for b in range(B)]
    x8s = [_stage(xb * X_SCALE, f8) for xb in xTs]
    xbs = [_stage(xb, bf) for xb in xTs]
    maps = []
    for c in range(NCORES):
        b, g = c // GROUP, c % GROUP
        cs = slice(g * CW, (g + 1) * CW)
        maps.append({
            "x8": x8s[b],
            "xT": xbs[b],
            "wq": _stage(Wq[:, cs] * W_SCALE, f8),
            "wk": _stage(Wk[:, cs] * W_SCALE, f8),
            "wv": _stage(Wv[:, cs], bf),
            "wo": _stage(np.ascontiguousarray(Wo[cs, :]), bf),
            "msk": msk,
            "vones": np.ones((128, NKC, HPC, 1), dtype=bf),
            "sel33": sel33,
        })
    return maps


def kernel(x, Wq, Wk, Wv, Wo, bo, _trace=False):
    x = np.asarray(x, dtype=np.float32)
    Wq, Wk, Wv, Wo, bo = (np.asarray(a, dtype=np.float32) for a in (Wq, Wk, Wv, Wo, bo))
    if "nc" not in _CACHE:
        _CACHE["nc"] = _build_bass()
    nc = _CACHE["nc"]
    res = run_bass_kernel_spmd(
        nc, _in_maps(x, Wq, Wk, Wv, Wo, bo), list(range(NCORES)), trace=_trace
    )
    out = np.zeros((B, S, D), dtype=np.float32)
    for b in range(B):
        acc = np.zeros((D, S), dtype=np.float32)
        for g in range(GROUP):
            acc += res.results[GROUP * b + g]["outT"].astype(np.float32)
        out[b] = acc.T + bo[None, :]
    if _trace:
        return out, res
    return out


# revision 30
# speedup vs baseline: 1.0227x; 1.0013x over previous
"""Distributed causal multi-head attention for Trainium2 (8 NeuronCores).

Problem (hardcoded): x[2, 2048, 1024], 16 heads, head_dim 64, causal
softmax(QK^T/8)V then out-proj with bias. f32 in/out.

Sharding: data parallel on batch (cores 0-3 -> batch 0, 4-7 -> batch 1),
tensor parallel on heads within each group of 4 (4 heads per core).
Each core:
  - computes Q^T,K^T via fp8(e4m3) DoubleRow matmuls (x pre-scaled by 8,
    Wq/Wk by 64 on the host; the 512^2 product scale is folded into the
    softmax exp scale), V in bf16
  - scores transposed S^T[k,q] = K Q^T so the softmax denominator comes out
    of the PE via an appended ones-column on V (no partition reductions)
  - exp without max-subtraction (scores are O(2), safe in fp32/bf16)
  - causal mask applied post-exp as a 0/1 bf16 multiply (DVE 2x mode)
  - ctx^T accumulated per q-chunk, normalized with 1/den partition-broadcast
  - row-parallel out-proj: each core computes the FULL-width partial
    outT_part[oc, q] = Wo[own 256 rows, oc]^T ctxT_own
No collectives: the host sums the 4 partial outputs per batch group
(standard row-parallel TP unshard), adds the bias, and transposes.

Attention/out-proj matmuls bf16 (fp32 PSUM accumulation); QK projections
fp8; partial outputs written bf16.
"""

import numpy as np
import ml_dtypes

from concourse import bass, bacc, mybir
from concourse import tile
from concourse.bass_utils import run_bass_kernel_spmd

BF16 = mybir.dt.bfloat16
F32 = mybir.dt.float32
FP8 = mybir.dt.float8e4
Act = mybir.ActivationFunctionType
DR = mybir.MatmulPerfMode.DoubleRow

B, S, D = 2, 2048, 1024
H, HD = 16, 64
NCORES = 8
GROUP = 4            # cores per batch group
HPC = H // GROUP     # 4 heads per core
CW = HPC * HD        # 256 ctx columns per core
QC = 512             # q-chunk width
KC = 128             # k-chunk width
NQ = S // QC         # 4
NKC = S // KC        # 16
KPQ = QC // KC       # 4 k-chunks per q-chunk
DCH = D // 128       # 8 contraction chunks of 128
CCH = CW // 128      # 2 own-ctx contraction chunks
OCH = D // 128       # 8 out-column chunks

X_SCALE = 8.0        # host pre-scale of x before fp8 quantization
W_SCALE = 64.0       # host pre-scale of Wq/Wk before fp8 quantization
# scores' = (512 q)·(512 k); exp(q·k/8) -> scale out the 512^2
EXP_SCALE = 0.125 / (X_SCALE * W_SCALE) ** 2

_CACHE = {}


def _build_bass(reps=1):
    nc = bacc.Bacc(
        "TRN2", target_bir_lowering=False, debug=False, num_devices=NCORES
    )

    # per-core external inputs, pre-staged by the host in SBUF layout
    # [partition, chunk, free] so each is a single large-descriptor DMA
    x8 = nc.declare_dram_parameter("x8", [128, DCH, S], FP8, isOutput=False)
    xT = nc.declare_dram_parameter("xT", [128, DCH, S], BF16, isOutput=False)
    wq = nc.declare_dram_parameter("wq", [128, DCH, CW], FP8, isOutput=False)
    wk = nc.declare_dram_parameter("wk", [128, DCH, CW], FP8, isOutput=False)
    wv = nc.declare_dram_parameter("wv", [128, DCH, CW], BF16, isOutput=False)
    wo = nc.declare_dram_parameter("wo", [128, CCH, D], BF16, isOutput=False)
    msk = nc.declare_dram_parameter("msk", [128, KPQ, QC], BF16, isOutput=False)
    vones = nc.declare_dram_parameter("vones", [128, NKC, HPC, 1], BF16, isOutput=False)
    # selector for den broadcast: bc[m,q] = sum_k sel33[k,m]*den_pair[k,q]
    sel33 = nc.declare_dram_parameter("sel33", [33, 128], BF16, isOutput=False)
    outT = nc.declare_dram_parameter("outT", [D, S], BF16, isOutput=True)

    with tile.TileContext(nc) as tc:
        with tc.tile_pool(name="persist", bufs=1) as pp:
            x8_sb = pp.tile([128, DCH, S], FP8, tag="x8_sb")
            xT_sb = pp.tile([128, DCH, S], BF16, tag="xT_sb")
            wq_sb = pp.tile([128, DCH, CW], FP8, tag="wq_sb")
            wk_sb = pp.tile([128, DCH, CW], FP8, tag="wk_sb")
            wv_sb = pp.tile([128, DCH, CW], BF16, tag="wv_sb")
            wo_sb = pp.tile([128, CCH, D], BF16, tag="wo_sb")
            msk_sb = pp.tile([128, KPQ, QC], BF16, tag="msk_sb")
            qT_sb = pp.tile([128, 2, S], BF16, tag="qT_sb")
            kT_sb = pp.tile([128, 2, S], BF16, tag="kT_sb")
            v_aug = pp.tile([128, NKC, HPC, HD + 1], BF16, tag="v_aug")
            ctxu0 = pp.tile([128, S], F32, tag="ctxu0")
            ctxu1 = pp.tile([128, S], F32, tag="ctxu1")
            ctxn0 = pp.tile([128, S], BF16, tag="ctxn0")
            ctxn1 = pp.tile([128, S], BF16, tag="ctxn1")
            # den per pair: head 2p at partition 0, head 2p+1 at partition
            # 32 (ACT writes must start at multiples of 32); rows 1-31 are
            # zeroed so the K=33 selector matmul can broadcast both heads
            # to output partitions 0-63 / 64-127 in one instruction
            den_pair = [pp.tile([33, S], BF16, tag=f"den{p}", name=f"den{p}")
                        for p in range(2)]
            sel_sb = pp.tile([33, 128], BF16, tag="sel_sb")
            ctxu_pair = [ctxu0, ctxu1]
            ctxn_pair = [ctxn0, ctxn1]
            # Pool engine (idle otherwise) zeroes the den scratch
            for p in range(2):
                nc.gpsimd.memset(den_pair[p][:], 0.0)

            # DMA order = first-use order. The first x8 window is split
            # per d-chunk so the very first projection chain unblocks
            # after wq + one small chunk; everything else streams behind.
            # All of x8 (fp8, 2MB) lands before xT (bf16, 4MB) so the QK
            # projections and all j=0 scores can run while V streams in.
            def _x8w(w):
                nc.sync.dma_start(
                    x8_sb[:, :, w * QC:(w + 1) * QC],
                    x8[:, :, w * QC:(w + 1) * QC],
                )

            def _xTw(w):
                nc.sync.dma_start(
                    xT_sb[:, :, w * QC:(w + 1) * QC],
                    xT[:, :, w * QC:(w + 1) * QC],
                )

            nc.sync.dma_start(wq_sb[:], wq[:])
            nc.sync.dma_start(wk_sb[:], wk[:])
            for c in range(DCH):
                nc.sync.dma_start(x8_sb[:, c, 0:QC], x8[:, c, 0:QC])
            nc.sync.dma_start(msk_sb[:], msk[:])
            for w in range(1, NQ):
                _x8w(w)
            nc.sync.dma_start(wv_sb[:], wv[:])
            # ones column of V_aug comes from the host: keeps the V
            # PSUM->SBUF copy to a single (PE) sync wait
            nc.sync.dma_start(v_aug[:, :, :, HD:HD + 1], vones[:])
            nc.sync.dma_start(sel_sb[:], sel33[:])
            for w in range(NQ):
                _xTw(w)
            nc.sync.dma_start(wo_sb[:], wo[:])

            def _emit_once():
                with tc.tile_pool(name="proj_ps", bufs=2, space="PSUM") as projp, \
                     tc.tile_pool(name="sc_ps", bufs=2, space="PSUM") as scp, \
                     tc.tile_pool(name="ctbc_ps", bufs=2, space="PSUM") as ctp, \
                     tc.tile_pool(name="es_pool", bufs=26) as esp, \
                     tc.tile_pool(name="out_sb", bufs=8) as outs, \
                     tc.tile_pool(name="norm", bufs=2) as np_pool:

                    def qk_round(j):
                        # Q,K projections for q/k-token window j, both
                        # pairs; fp8 DoubleRow: chunk pairs -> K=256
                        for pair in range(2):
                            for w_sb, dst in ((wq_sb, qT_sb), (wk_sb, kT_sb)):
                                ps = projp.tile([128, QC], F32, tag="proj")
                                for c in range(0, DCH, 2):
                                    nc.tensor.matmul(
                                        ps[:],
                                        w_sb[:, c:c + 2, pair * 128:(pair + 1) * 128],
                                        x8_sb[:, c:c + 2, j * QC:(j + 1) * QC],
                                        start=(c == 0),
                                        stop=(c == DCH - 2),
                                        perf_mode=DR,
                                    )
                                nc.vector.tensor_copy(
                                    dst[:, pair, j * QC:(j + 1) * QC], ps[:]
                                )

                    def v_round(w):
                        # V for token chunks 4w..4w+3, both pairs
                        for t in range(4 * w, 4 * w + 4):
                            for pair in range(2):
                                ps = projp.tile([128, QC], F32, tag="proj")
                                for c in range(DCH):
                                    nc.tensor.matmul(
                                        ps[:, 0:128],
                                        xT_sb[:, c, t * 128:(t + 1) * 128],
                                        wv_sb[:, c, pair * 128:(pair + 1) * 128],
                                        start=(c == 0),
                                        stop=(c == DCH - 1),
                                    )
                                nc.vector.tensor_copy(
                                    v_aug[:, t, 2 * pair:2 * pair + 2, 0:HD],
                                    ps[:, 0:128].rearrange("p (h w) -> p h w", h=2),
                                )

                    def scores_exp(h, j):
                        """S^T then exp (+ causal masking) for q-chunk j of
                        head h. Off-band k-chunks (fully below the diagonal)
                        get full-width matmuls; the 4-chunk diagonal band
                        uses shrinking q-windows (exact block causality)
                        with a 128-wide triangle mask per chunk. Returns the
                        es tiles (off-band pairs + band tiles 1 and 2)."""
                        pair, hh = h // 2, h % 2
                        row = hh * 64
                        qs = slice(j * QC, (j + 1) * QC)
                        es_tiles = []
                        # off-band: k-chunks 0 .. 4j-1, two per PSUM tile
                        for c0 in range(0, 4 * j, 2):
                            st = scp.tile([128, 2, QC], F32, tag="st")
                            for i in range(2):
                                c = c0 + i
                                nc.tensor.matmul(
                                    st[:, i, :],
                                    kT_sb[row:row + 64, pair, c * KC:(c + 1) * KC],
                                    qT_sb[row:row + 64, pair, qs],
                                    start=True, stop=True,
                                )
                            es = esp.tile([128, 2, QC], BF16, tag="es")
                            nc.scalar.activation(es[:], st[:], Act.Exp,
                                                 scale=EXP_SCALE)
                            es_tiles.append(es)
                        # diagonal band: k-chunks 4j+r, q-window [128r, 512)
                        # packed as two tiles; sub-window starts snapped so
                        # each exp is one full-AP instruction (the unwritten
                        # PSUM slivers are exp'd but never read)
                        win = [0, KC, 2 * KC, 3 * KC]
                        for ti in range(2):
                            st = scp.tile([128, 2, QC], F32, tag="st")
                            for i in range(2):
                                r = 2 * ti + i
                                w0 = win[r]
                                nc.tensor.matmul(
                                    st[:, i, w0:QC],
                                    kT_sb[row:row + 64, pair,
                                          (4 * j + r) * KC:(4 * j + r + 1) * KC],
                                    qT_sb[row:row + 64, pair,
                                          j * QC + w0:(j + 1) * QC],
                                    start=True, stop=True,
                                )
                            es = esp.tile([128, 2, QC], BF16, tag="es")
                            lo = win[2 * ti]
                            nc.scalar.activation(es[:, :, lo:QC],
                                                 st[:, :, lo:QC],
                                                 Act.Exp, scale=EXP_SCALE)
                            # triangle mask on the leading 128 q of each
                            # chunk's window
                            for i in range(2):
                                r = 2 * ti + i
                                w0 = win[r]
                                nc.vector.tensor_mul(
                                    es[:, i, w0:w0 + KC], es[:, i, w0:w0 + KC],
                                    msk_sb[:, r, w0:w0 + KC],
                                )
                            es_tiles.append(es)
                        return es_tiles

                    def ctx_acc(h, j, es_tiles):
                        """attn@V accumulation + write-back for (h, j)."""
                        pair, hh = h // 2, h % 2
                        row = hh * 64
                        qs = slice(j * QC, (j + 1) * QC)
                        win = [0, KC, 2 * KC, 3 * KC]
                        ct = ctp.tile([HD + 1, QC], F32, tag="ct")
                        for c in range(4 * j):
                            nc.tensor.matmul(
                                ct[:],
                                v_aug[:, c, h, :],
                                es_tiles[c // 2][:, c % 2, :],
                                start=(c == 0),
                                stop=False,
                            )
                        for r in range(4):
                            w0 = win[r]
                            nc.tensor.matmul(
                                ct[:, w0:QC],
                                v_aug[:, 4 * j + r, h, :],
                                es_tiles[2 * j + r // 2][:, r % 2, w0:QC],
                                start=(j == 0 and r == 0),
                                stop=(r == 3),
                            )
                        nc.vector.tensor_copy(
                            ctxu_pair[pair][row:row + 64, qs], ct[0:HD, :]
                        )
                        if h == 3 and j == NQ - 1:
                            # final head: ACT is idle by now and the DVE is
                            # busy with the ctx copy — take the den copy off
                            # the critical chain (partition 32: ACT writes
                            # must start at multiples of 32)
                            nc.scalar.activation(
                                den_pair[pair][hh * 32:hh * 32 + 1, qs],
                                ct[HD:HD + 1, :], Act.Identity,
                            )
                        else:
                            nc.vector.tensor_copy(
                                den_pair[pair][hh * 32:hh * 32 + 1, qs],
                                ct[HD:HD + 1, :],
                            )

                    def norm(pair, j):
                        # den[pair][:, qs] complete once both heads of the
                        # pair finished ctx for q-chunk j; single fused
                        # divide (no reciprocal round-trip)
                        qs = slice(j * QC, (j + 1) * QC)
                        bc = ctp.tile([128, QC], F32, tag="ct")
                        nc.tensor.matmul(
                            bc[:], sel_sb[:], den_pair[pair][:, qs],
                            start=True, stop=True,
                        )
                        rb = np_pool.tile([128, QC], F32, tag="rb")
                        nc.vector.reciprocal(rb[:], bc[:])
                        nc.vector.tensor_mul(
                            ctxn_pair[pair][:, qs],
                            ctxu_pair[pair][:, qs], rb[:],
                        )

                    # Interleaved emission. QK projections and all j=0
                    # scores depend only on x8 (fp8, lands first), so they
                    # run while the larger bf16 x for V is still streaming;
                    # exp starts ~15us earlier than a phase-ordered kernel.
                    # The attention pipeline emits scores of the next chunk
                    # before ctx of the previous one so the PE has queued
                    # work while ACT runs exp; each (pair, j) normalizes as
                    # soon as its den is complete.
                    def out_round(j, last=False):
                        # row-parallel out-proj for q-chunk j:
                        # outT_part[oc, q] = Wo[own, oc]^T ctxn_own (bias on
                        # host). PSUM alternates between the projection pool
                        # (dead once the v_rounds finish) and a scores-pool
                        # slice so the matmuls can run ~2 copies ahead.
                        # While interleaved into late attention (ACT is
                        # exp-critical there) all copies go to DVE with the
                        # DMA issued from the idle Pool queue; the final
                        # round alternates ACT/DVE.
                        qs = slice(j * QC, (j + 1) * QC)
                        ps2 = None
                        for o in range(OCH):
                            if not last:
                                ps = projp.tile([128, QC], F32, tag="proj")
                            elif o % 4 < 2:
                                # final round: attention PSUM is free, so
                                # widen the accumulator set (6 in flight)
                                # to keep the matmuls ahead of the copies
                                if o % 4 == 0:
                                    ps2 = scp.tile([128, 2, QC], F32,
                                                   tag="st", name=f"eps{o}")
                                ps = ps2[:, o % 4, :]
                            else:
                                ps = projp.tile([128, QC], F32, tag="proj")
                            for c in range(CCH):
                                nc.tensor.matmul(
                                    ps[:],
                                    wo_sb[:, c, o * 128:(o + 1) * 128],
                                    ctxn_pair[c][:, qs],
                                    start=(c == 0),
                                    stop=(c == CCH - 1),
                                )
                            # while interleaved into late attention, ACT is
                            # exp-critical: keep copies off it (DVE + Pool
                            # DMA); the final round alternates ACT/DVE
                            ot = outs.tile([128, QC], BF16, tag="ot")
                            if last and o % 2 == 0:
                                nc.scalar.activation(ot[:], ps[:], Act.Identity)
                                nc.scalar.dma_start(
                                    outT[o * 128:(o + 1) * 128, qs], ot[:]
                                )
                            else:
                                nc.vector.tensor_copy(ot[:], ps[:])
                                nc.gpsimd.dma_start(
                                    outT[o * 128:(o + 1) * 128, qs], ot[:]
                                )

                    qk_round(0)
                    es00 = scores_exp(0, 0)
                    es10 = scores_exp(1, 0)
                    qk_round(1)
                    es20 = scores_exp(2, 0)
                    es30 = scores_exp(3, 0)
                    qk_round(2)
                    qk_round(3)
                    v_round(0)
                    ctx_acc(0, 0, es00)
                    ctx_acc(1, 0, es10)
                    norm(0, 0)
                    es01 = scores_exp(0, 1)
                    v_round(1)
                    ctx_acc(2, 0, es20)
                    ctx_acc(3, 0, es30)
                    norm(1, 0)
                    es11 = scores_exp(1, 1)
                    ctx_acc(0, 1, es01)
                    v_round(2)
                    es21 = scores_exp(2, 1)
                    ctx_acc(1, 1, es11)
                    norm(0, 1)
                    v_round(3)
                    es31 = scores_exp(3, 1)
                    ctx_acc(2, 1, es21)
                    # depth-2 pipeline into j=2 so ACT always has exp queued
                    es02 = scores_exp(0, 2)
                    ctx_acc(3, 1, es31)
                    norm(1, 1)
                    es12 = scores_exp(1, 2)
                    ctx_acc(0, 2, es02)
                    es22 = scores_exp(2, 2)
                    ctx_acc(1, 2, es12)
                    norm(0, 2)
                    es32 = scores_exp(3, 2)
                    ctx_acc(2, 2, es22)
                    es03 = scores_exp(0, 3)
                    ctx_acc(3, 2, es32)
                    norm(1, 2)
                    es13 = scores_exp(1, 3)
                    ctx_acc(0, 3, es03)
                    out_round(0)
                    es23 = scores_exp(2, 3)
                    ctx_acc(1, 3, es13)
                    norm(0, 3)
                    es33 = scores_exp(3, 3)
                    out_round(1)
                    ctx_acc(2, 3, es23)
                    out_round(2)
                    ctx_acc(3, 3, es33)
                    norm(1, 3)
                    out_round(3, last=True)

            for _rep in range(reps):
                _emit_once()
    nc.compile()
    return nc


def _causal_mask():
    # msk[kp, r, qf] = 1 where (r*128 + kp) <= qf else 0  (keep k <= q)
    kp = np.arange(128)[:, None, None]
    r = np.arange(KPQ)[None, :, None]
    qf = np.arange(QC)[None, None, :]
    return (r * 128 + kp <= qf).astype(ml_dtypes.bfloat16)


def _stage(a, dtype, pdim=128):
    """[pdim*n, free...] -> contiguous [pdim, n, free...]"""
    n = a.shape[0] // pdim
    out = a.reshape((n, pdim) + a.shape[1:]).transpose(
        (1, 0) + tuple(range(2, a.ndim + 1))
    )
    return np.ascontiguousarray(out.astype(dtype))


def _in_maps(x, Wq, Wk, Wv, Wo, bo):
    bf = ml_dtypes.bfloat16
    f8 = ml_dtypes.float8_e4m3
    msk = _causal_mask()
    sel33 = np.zeros((33, 128), dtype=bf)
    sel33[0, 0:64] = 1.0
    sel33[32, 64:128] = 1.0
    xTs = [np.ascontiguousarray(x[b].T) for b in range(B)]
    x8s = [_stage(xb * X_SCALE, f8) for xb in xTs]
    xbs = [_stage(xb, bf) for xb in xTs]
    maps = []
    for c in range(NCORES):
        b, g = c // GROUP, c % GROUP
        cs = slice(g * CW, (g + 1) * CW)
        maps.append({
            "x8": x8s[b],
            "xT": xbs[b],
            "wq": _stage(Wq[:, cs] * W_SCALE, f8),
            "wk": _stage(Wk[:, cs] * W_SCALE, f8),
            "wv": _stage(Wv[:, cs], bf),
            "wo": _stage(np.ascontiguousarray(Wo[cs, :]), bf),
            "msk": msk,
            "vones": np.ones((128, NKC, HPC, 1), dtype=bf),
            "sel33": sel33,
        })
    return maps


def kernel(x, Wq, Wk, Wv, Wo, bo, _trace=False):
    x = np.asarray(x, dtype=np.float32)
    Wq, Wk, Wv, Wo, bo = (np.asarray(a, dtype=np.float32) for a in (Wq, Wk, Wv, Wo, bo))
    if "nc" not in _CACHE:
        _CACHE["nc"] = _build_bass()
    nc = _CACHE["nc"]
    res = run_bass_kernel_spmd(
        nc, _in_maps(x, Wq, Wk, Wv, Wo, bo), list(range(NCORES)), trace=_trace
    )
    out = np.zeros((B, S, D), dtype=np.float32)
    for b in range(B):
        acc = np.zeros((D, S), dtype=np.float32)
        for g in range(GROUP):
            acc += res.results[GROUP * b + g]["outT"].astype(np.float32)
        out[b] = acc.T + bo[None, :]
    if _trace:
        return out, res
    return out


# revision 33
# speedup vs baseline: 1.0319x; 1.0090x over previous
"""Distributed causal multi-head attention for Trainium2 (8 NeuronCores).

Problem (hardcoded): x[2, 2048, 1024], 16 heads, head_dim 64, causal
softmax(QK^T/8)V then out-proj with bias. f32 in/out.

Sharding: data parallel on batch (cores 0-3 -> batch 0, 4-7 -> batch 1),
tensor parallel on heads within each group of 4 (4 heads per core).
Each core:
  - computes Q^T,K^T via fp8(e4m3) DoubleRow matmuls (x pre-scaled by 8,
    Wq/Wk by 64 on the host; the 512^2 product scale is folded into the
    softmax exp scale), V in bf16
  - scores transposed S^T[k,q] = K Q^T so the softmax denominator comes out
    of the PE via an appended ones-column on V (no partition reductions)
  - exp without max-subtraction (scores are O(2), safe in fp32/bf16)
  - causal mask applied post-exp as a 0/1 bf16 multiply (DVE 2x mode)
  - ctx^T accumulated per q-chunk, normalized with 1/den partition-broadcast
  - row-parallel out-proj: each core computes the FULL-width partial
    outT_part[oc, q] = Wo[own 256 rows, oc]^T ctxT_own
No collectives: the host sums the 4 partial outputs per batch group
(standard row-parallel TP unshard), adds the bias, and transposes.

Attention/out-proj matmuls bf16 (fp32 PSUM accumulation); QK projections
fp8; partial outputs written bf16.
"""

import numpy as np
import ml_dtypes

from concourse import bass, bacc, mybir
from concourse import tile
from concourse.bass_utils import run_bass_kernel_spmd

BF16 = mybir.dt.bfloat16
F32 = mybir.dt.float32
FP8 = mybir.dt.float8e4
Act = mybir.ActivationFunctionType
DR = mybir.MatmulPerfMode.DoubleRow

B, S, D = 2, 2048, 1024
H, HD = 16, 64
NCORES = 8
GROUP = 4            # cores per batch group
HPC = H // GROUP     # 4 heads per core
CW = HPC * HD        # 256 ctx columns per core
QC = 512             # q-chunk width
KC = 128             # k-chunk width
NQ = S // QC         # 4
NKC = S // KC        # 16
KPQ = QC // KC       # 4 k-chunks per q-chunk
DCH = D // 128       # 8 contraction chunks of 128
CCH = CW // 128      # 2 own-ctx contraction chunks
OCH = D // 128       # 8 out-column chunks

X_SCALE = 8.0        # host pre-scale of x before fp8 quantization
W_SCALE = 64.0       # host pre-scale of Wq/Wk before fp8 quantization
# scores' = (512 q)·(512 k); exp(q·k/8) -> scale out the 512^2
EXP_SCALE = 0.125 / (X_SCALE * W_SCALE) ** 2

_CACHE = {}


def _build_bass(reps=1):
    nc = bacc.Bacc(
        "TRN2", target_bir_lowering=False, debug=False, num_devices=NCORES
    )

    # per-core external inputs, pre-staged by the host in SBUF layout
    # [partition, chunk, free] so each is a single large-descriptor DMA
    x8 = nc.declare_dram_parameter("x8", [128, DCH, S], FP8, isOutput=False)
    xT = nc.declare_dram_parameter("xT", [128, DCH, S], BF16, isOutput=False)
    wq = nc.declare_dram_parameter("wq", [128, DCH, CW], FP8, isOutput=False)
    wk = nc.declare_dram_parameter("wk", [128, DCH, CW], FP8, isOutput=False)
    wv = nc.declare_dram_parameter("wv", [128, DCH, CW], BF16, isOutput=False)
    wo = nc.declare_dram_parameter("wo", [128, CCH, D], BF16, isOutput=False)
    msk = nc.declare_dram_parameter("msk", [128, KPQ, QC], BF16, isOutput=False)
    # selector for den broadcast: bc[m,q] = sum_k sel33[k,m]*den_pair[k,q]
    sel33 = nc.declare_dram_parameter("sel33", [33, 128], BF16, isOutput=False)
    outT = nc.declare_dram_parameter("outT", [D, S], BF16, isOutput=True)

    with tile.TileContext(nc) as tc:
        with tc.tile_pool(name="persist", bufs=1) as pp:
            x8_sb = pp.tile([128, DCH, S], FP8, tag="x8_sb")
            xT_sb = pp.tile([128, DCH, S], BF16, tag="xT_sb")
            wq_sb = pp.tile([128, DCH, CW], FP8, tag="wq_sb")
            wk_sb = pp.tile([128, DCH, CW], FP8, tag="wk_sb")
            wv_sb = pp.tile([128, DCH, CW], BF16, tag="wv_sb")
            wo_sb = pp.tile([128, CCH, D], BF16, tag="wo_sb")
            msk_sb = pp.tile([128, KPQ, QC], BF16, tag="msk_sb")
            qT_sb = pp.tile([128, 2, S], BF16, tag="qT_sb")
            kT_sb = pp.tile([128, 2, S], BF16, tag="kT_sb")
            v_aug = pp.tile([128, NKC, HPC, HD + 1], BF16, tag="v_aug")
            ctxu0 = pp.tile([128, S], F32, tag="ctxu0")
            ctxu1 = pp.tile([128, S], F32, tag="ctxu1")
            ctxn0 = pp.tile([128, S], BF16, tag="ctxn0")
            ctxn1 = pp.tile([128, S], BF16, tag="ctxn1")
            # den per pair: head 2p at partition 0, head 2p+1 at partition
            # 32 (ACT writes must start at multiples of 32); rows 1-31 are
            # zeroed so the K=33 selector matmul can broadcast both heads
            # to output partitions 0-63 / 64-127 in one instruction
            den_pair = [pp.tile([33, S], BF16, tag=f"den{p}", name=f"den{p}")
                        for p in range(2)]
            sel_sb = pp.tile([33, 128], BF16, tag="sel_sb")
            ctxu_pair = [ctxu0, ctxu1]
            ctxn_pair = [ctxn0, ctxn1]
            # Pool engine (idle otherwise) zeroes the den scratch
            for p in range(2):
                nc.gpsimd.memset(den_pair[p][:], 0.0)

            # DMA order = first-use order. The first x8 window is split
            # per d-chunk so the very first projection chain unblocks
            # after wq + one small chunk; everything else streams behind.
            # All of x8 (fp8, 2MB) lands before xT (bf16, 4MB) so the QK
            # projections and all j=0 scores can run while V streams in.
            def _x8w(w):
                nc.sync.dma_start(
                    x8_sb[:, :, w * QC:(w + 1) * QC],
                    x8[:, :, w * QC:(w + 1) * QC],
                )

            def _xTw(w):
                nc.sync.dma_start(
                    xT_sb[:, :, w * QC:(w + 1) * QC],
                    xT[:, :, w * QC:(w + 1) * QC],
                )

            # ones column of V_aug via the idle Pool engine: keeps the V
            # PSUM->SBUF copy to a single (PE) sync wait
            nc.gpsimd.memset(v_aug[:, :, :, HD:HD + 1], 1.0)
            nc.sync.dma_start(wq_sb[:], wq[:])
            nc.sync.dma_start(wk_sb[:], wk[:])
            for c in range(DCH):
                nc.sync.dma_start(x8_sb[:, c, 0:QC], x8[:, c, 0:QC])
            for w in range(1, NQ):
                _x8w(w)
            # msk is first consumed by the j=0 mask muls (~15us in): keep it
            # out of the latency-critical x8 stretch
            nc.sync.dma_start(msk_sb[:], msk[:])
            nc.sync.dma_start(wv_sb[:], wv[:])
            nc.sync.dma_start(sel_sb[:], sel33[:])
            for w in range(NQ):
                _xTw(w)
            nc.sync.dma_start(wo_sb[:], wo[:])

            def _emit_once():
                with tc.tile_pool(name="proj_ps", bufs=2, space="PSUM") as projp, \
                     tc.tile_pool(name="sc_ps", bufs=2, space="PSUM") as scp, \
                     tc.tile_pool(name="ctbc_ps", bufs=2, space="PSUM") as ctp, \
                     tc.tile_pool(name="es_pool", bufs=26) as esp, \
                     tc.tile_pool(name="out_sb", bufs=8) as outs, \
                     tc.tile_pool(name="norm", bufs=2) as np_pool:

                    def qk_round(j):
                        # Q,K projections for q/k-token window j, both
                        # pairs; fp8 DoubleRow: chunk pairs -> K=256
                        for pair in range(2):
                            for w_sb, dst in ((wq_sb, qT_sb), (wk_sb, kT_sb)):
                                ps = projp.tile([128, QC], F32, tag="proj")
                                for c in range(0, DCH, 2):
                                    nc.tensor.matmul(
                                        ps[:],
                                        w_sb[:, c:c + 2, pair * 128:(pair + 1) * 128],
                                        x8_sb[:, c:c + 2, j * QC:(j + 1) * QC],
                                        start=(c == 0),
                                        stop=(c == DCH - 2),
                                        perf_mode=DR,
                                    )
                                nc.vector.tensor_copy(
                                    dst[:, pair, j * QC:(j + 1) * QC], ps[:]
                                )

                    def v_round(w):
                        # V for token chunks 4w..4w+3, both pairs
                        for t in range(4 * w, 4 * w + 4):
                            for pair in range(2):
                                ps = projp.tile([128, QC], F32, tag="proj")
                                for c in range(DCH):
                                    nc.tensor.matmul(
                                        ps[:, 0:128],
                                        xT_sb[:, c, t * 128:(t + 1) * 128],
                                        wv_sb[:, c, pair * 128:(pair + 1) * 128],
                                        start=(c == 0),
                                        stop=(c == DCH - 1),
                                    )
                                nc.vector.tensor_copy(
                                    v_aug[:, t, 2 * pair:2 * pair + 2, 0:HD],
                                    ps[:, 0:128].rearrange("p (h w) -> p h w", h=2),
                                )

                    def scores_exp(h, j):
                        """S^T then exp (+ causal masking) for q-chunk j of
                        head h. Off-band k-chunks (fully below the diagonal)
                        get full-width matmuls; the 4-chunk diagonal band
                        uses shrinking q-windows (exact block causality)
                        with a 128-wide triangle mask per chunk. Returns the
                        es tiles (off-band pairs + band tiles 1 and 2)."""
                        pair, hh = h // 2, h % 2
                        row = hh * 64
                        qs = slice(j * QC, (j + 1) * QC)
                        es_tiles = []
                        # off-band: k-chunks 0 .. 4j-1, two per PSUM tile
                        for c0 in range(0, 4 * j, 2):
                            st = scp.tile([128, 2, QC], F32, tag="st")
                            for i in range(2):
                                c = c0 + i
                                nc.tensor.matmul(
                                    st[:, i, :],
                                    kT_sb[row:row + 64, pair, c * KC:(c + 1) * KC],
                                    qT_sb[row:row + 64, pair, qs],
                                    start=True, stop=True,
                                )
                            es = esp.tile([128, 2, QC], BF16, tag="es")
                            nc.scalar.activation(es[:], st[:], Act.Exp,
                                                 scale=EXP_SCALE)
                            es_tiles.append(es)
                        # diagonal band: k-chunks 4j+r, q-window [128r, 512)
                        # packed as two tiles; sub-window starts snapped so
                        # each exp is one full-AP instruction (the unwritten
                        # PSUM slivers are exp'd but never read)
                        win = [0, KC, 2 * KC, 3 * KC]
                        for ti in range(2):
                            st = scp.tile([128, 2, QC], F32, tag="st")
                            for i in range(2):
                                r = 2 * ti + i
                                w0 = win[r]
                                nc.tensor.matmul(
                                    st[:, i, w0:QC],
                                    kT_sb[row:row + 64, pair,
                                          (4 * j + r) * KC:(4 * j + r + 1) * KC],
                                    qT_sb[row:row + 64, pair,
                                          j * QC + w0:(j + 1) * QC],
                                    start=True, stop=True,
                                )
                            es = esp.tile([128, 2, QC], BF16, tag="es")
                            lo = win[2 * ti]
                            nc.scalar.activation(es[:, :, lo:QC],
                                                 st[:, :, lo:QC],
                                                 Act.Exp, scale=EXP_SCALE)
                            # triangle mask on the leading 128 q of each
                            # chunk's window
                            for i in range(2):
                                r = 2 * ti + i
                                w0 = win[r]
                                nc.vector.tensor_mul(
                                    es[:, i, w0:w0 + KC], es[:, i, w0:w0 + KC],
                                    msk_sb[:, r, w0:w0 + KC],
                                )
                            es_tiles.append(es)
                        return es_tiles

                    def ctx_acc(h, j, es_tiles):
                        """attn@V accumulation + write-back for (h, j)."""
                        pair, hh = h // 2, h % 2
                        row = hh * 64
                        qs = slice(j * QC, (j + 1) * QC)
                        win = [0, KC, 2 * KC, 3 * KC]
                        ct = ctp.tile([HD + 1, QC], F32, tag="ct")
                        for c in range(4 * j):
                            nc.tensor.matmul(
                                ct[:],
                                v_aug[:, c, h, :],
                                es_tiles[c // 2][:, c % 2, :],
                                start=(c == 0),
                                stop=False,
                            )
                        for r in range(4):
                            w0 = win[r]
                            nc.tensor.matmul(
                                ct[:, w0:QC],
                                v_aug[:, 4 * j + r, h, :],
                                es_tiles[2 * j + r // 2][:, r % 2, w0:QC],
                                start=(j == 0 and r == 0),
                                stop=(r == 3),
                            )
                        nc.vector.tensor_copy(
                            ctxu_pair[pair][row:row + 64, qs], ct[0:HD, :]
                        )
                        if h == 3 and j == NQ - 1:
                            # final head: ACT is idle by now and the DVE is
                            # busy with the ctx copy — take the den copy off
                            # the critical chain (partition 32: ACT writes
                            # must start at multiples of 32)
                            nc.scalar.activation(
                                den_pair[pair][hh * 32:hh * 32 + 1, qs],
                                ct[HD:HD + 1, :], Act.Identity,
                            )
                        else:
                            nc.vector.tensor_copy(
                                den_pair[pair][hh * 32:hh * 32 + 1, qs],
                                ct[HD:HD + 1, :],
                            )

                    def norm(pair, j):
                        # den[pair][:, qs] complete once both heads of the
                        # pair finished ctx for q-chunk j; single fused
                        # divide (no reciprocal round-trip)
                        qs = slice(j * QC, (j + 1) * QC)
                        bc = ctp.tile([128, QC], F32, tag="ct")
                        nc.tensor.matmul(
                            bc[:], sel_sb[:], den_pair[pair][:, qs],
                            start=True, stop=True,
                        )
                        rb = np_pool.tile([128, QC], F32, tag="rb")
                        nc.vector.reciprocal(rb[:], bc[:])
                        nc.vector.tensor_mul(
                            ctxn_pair[pair][:, qs],
                            ctxu_pair[pair][:, qs], rb[:],
                        )

                    # Interleaved emission. QK projections and all j=0
                    # scores depend only on x8 (fp8, lands first), so they
                    # run while the larger bf16 x for V is still streaming;
                    # exp starts ~15us earlier than a phase-ordered kernel.
                    # The attention pipeline emits scores of the next chunk
                    # before ctx of the previous one so the PE has queued
                    # work while ACT runs exp; each (pair, j) normalizes as
                    # soon as its den is complete.
                    def out_round(j, last=False):
                        # row-parallel out-proj for q-chunk j:
                        # outT_part[oc, q] = Wo[own, oc]^T ctxn_own (bias on
                        # host). PSUM alternates between the projection pool
                        # (dead once the v_rounds finish) and a scores-pool
                        # slice so the matmuls can run ~2 copies ahead.
                        # While interleaved into late attention (ACT is
                        # exp-critical there) all copies go to DVE with the
                        # DMA issued from the idle Pool queue; the final
                        # round alternates ACT/DVE.
                        qs = slice(j * QC, (j + 1) * QC)
                        ps2 = None
                        for o in range(OCH):
                            if not last:
                                ps = projp.tile([128, QC], F32, tag="proj")
                            elif o % 4 < 2:
                                # final round: attention PSUM is free, so
                                # widen the accumulator set (6 in flight)
                                # to keep the matmuls ahead of the copies
                                if o % 4 == 0:
                                    ps2 = scp.tile([128, 2, QC], F32,
                                                   tag="st", name=f"eps{o}")
                                ps = ps2[:, o % 4, :]
                            else:
                                ps = projp.tile([128, QC], F32, tag="proj")
                            for c in range(CCH):
                                nc.tensor.matmul(
                                    ps[:],
                                    wo_sb[:, c, o * 128:(o + 1) * 128],
                                    ctxn_pair[c][:, qs],
                                    start=(c == 0),
                                    stop=(c == CCH - 1),
                                )
                            # while interleaved into late attention, ACT is
                            # exp-critical: keep copies off it (DVE + Pool
                            # DMA); the final round alternates ACT/DVE
                            ot = outs.tile([128, QC], BF16, tag="ot")
                            if last and o % 2 == 0:
                                nc.scalar.activation(ot[:], ps[:], Act.Identity)
                                nc.scalar.dma_start(
                                    outT[o * 128:(o + 1) * 128, qs], ot[:]
                                )
                            else:
                                nc.vector.tensor_copy(ot[:], ps[:])
                                nc.gpsimd.dma_start(
                                    outT[o * 128:(o + 1) * 128, qs], ot[:]
                                )

                    qk_round(0)
                    es00 = scores_exp(0, 0)
                    es10 = scores_exp(1, 0)
                    qk_round(1)
                    es20 = scores_exp(2, 0)
                    es30 = scores_exp(3, 0)
                    qk_round(2)
                    qk_round(3)
                    v_round(0)
                    ctx_acc(0, 0, es00)
                    ctx_acc(1, 0, es10)
                    norm(0, 0)
                    es01 = scores_exp(0, 1)
                    v_round(1)
                    ctx_acc(2, 0, es20)
                    ctx_acc(3, 0, es30)
                    norm(1, 0)
                    es11 = scores_exp(1, 1)
                    ctx_acc(0, 1, es01)
                    v_round(2)
                    es21 = scores_exp(2, 1)
                    ctx_acc(1, 1, es11)
                    norm(0, 1)
                    v_round(3)
                    es31 = scores_exp(3, 1)
                    ctx_acc(2, 1, es21)
                    # depth-2 pipeline into j=2 so ACT always has exp queued
                    es02 = scores_exp(0, 2)
                    ctx_acc(3, 1, es31)
                    norm(1, 1)
                    es12 = scores_exp(1, 2)
                    ctx_acc(0, 2, es02)
                    es22 = scores_exp(2, 2)
                    ctx_acc(1, 2, es12)
                    norm(0, 2)
                    es32 = scores_exp(3, 2)
                    ctx_acc(2, 2, es22)
                    es03 = scores_exp(0, 3)
                    ctx_acc(3, 2, es32)
                    norm(1, 2)
                    es13 = scores_exp(1, 3)
                    ctx_acc(0, 3, es03)
                    out_round(0)
                    es23 = scores_exp(2, 3)
                    ctx_acc(1, 3, es13)
                    norm(0, 3)
                    es33 = scores_exp(3, 3)
                    out_round(1)
                    ctx_acc(2, 3, es23)
                    out_round(2)
                    ctx_acc(3, 3, es33)
                    norm(1, 3)
                    out_round(3, last=True)

            for _rep in range(reps):
                _emit_once()
    nc.compile()
    return nc


def _causal_mask():
    # msk[kp, r, qf] = 1 where (r*128 + kp) <= qf else 0  (keep k <= q)
    kp = np.arange(128)[:, None, None]
    r = np.arange(KPQ)[None, :, None]
    qf = np.arange(QC)[None, None, :]
    return (r * 128 + kp <= qf).astype(ml_dtypes.bfloat16)


def _stage(a, dtype, pdim=128):
    """[pdim*n, free...] -> contiguous [pdim, n, free...]"""
    n = a.shape[0] // pdim
    out = a.reshape((n, pdim) + a.shape[1:]).transpose(
        (1, 0) + tuple(range(2, a.ndim + 1))
    )
    return np.ascontiguousarray(out.astype(dtype))


def _in_maps(x, Wq, Wk, Wv, Wo, bo):
    bf = ml_dtypes.bfloat16
    f8 = ml_dtypes.float8_e4m3
    msk = _causal_mask()
    sel33 = np.zeros((33, 128), dtype=bf)
    sel33[0, 0:64] = 1.0
    sel33[32, 64:128] = 1.0
    xTs = [np.ascontiguousarray(x[b].T) for b in range(B)]
    x8s = [_stage(xb * X_SCALE, f8) for xb in xTs]
    xbs = [_stage(xb, bf) for xb in xTs]
    maps = []
    for c in range(NCORES):
        b, g = c // GROUP, c % GROUP
        cs = slice(g * CW, (g + 1) * CW)
        maps.append({
            "x8": x8s[b],
            "xT": xbs[b],
            "wq": _stage(Wq[:, cs] * W_SCALE, f8),
            "wk": _stage(Wk[:, cs] * W_SCALE, f8),
            "wv": _stage(Wv[:, cs], bf),
            "wo": _stage(np.ascontiguousarray(Wo[cs, :]), bf),
            "msk": msk,
            "sel33": sel33,
        })
    return maps


def kernel(x, Wq, Wk, Wv, Wo, bo, _trace=False):
    x = np.asarray(x, dtype=np.float32)
    Wq, Wk, Wv, Wo, bo = (np.asarray(a, dtype=np.float32) for a in (Wq, Wk, Wv, Wo, bo))
    if "nc" not in _CACHE:
        _CACHE["nc"] = _build_bass()
    nc = _CACHE["nc"]
    res = run_bass_kernel_spmd(
        nc, _in_maps(x, Wq, Wk, Wv, Wo, bo), list(range(NCORES)), trace=_trace
    )
    out = np.zeros((B, S, D), dtype=np.float32)
    for b in range(B):
        acc = np.zeros((D, S), dtype=np.float32)
        for g in range(GROUP):
            acc += res.results[GROUP * b + g]["outT"].astype(np.float32)
        out[b] = acc.T + bo[None, :]
    if _trace:
        return out, res
    return out
